# revision 12
# baseline (speedup 1.0000x reference)
"""Grouped-Query Attention (B=2, S=2048, D=2048, H=32, KV=8, HD=64) on 8 TRN2
NeuronCores, tensor-parallel over KV-head groups (1 KV head + 4 Q heads per
core), with host-side shard/gather.

All matmul operands are fp16 (PE streams 1 cyc/row; PSUM accumulation fp32),
elementwise RoPE/softmax math fp32 where it reads PSUM.

Per-core dataflow (activations kept feature-on-partitions so every matmul
contracts over the partition dim with no on-device transposition of x):

  phase 1  QKV projection + RoPE
    xT d-tiles stream in batches of 4 (one DMA each) -> psum: qa/qb (S pair),
    kv (PV pair); RoPE on DVE from PSUM; Q written to qpack[0:64, h, tok]
    (rows 64:128 stay zero -- they face zero K-weights in the score matmul);
    K written to krotz rows 0:64 (rows 64:128 zero); V transposed back to
    natural [tok, hd] via PE transpose, + a ones column (softmax denominator).
  phase 2  attention per (batch, q-tile 512, head-pair), causal-block-skipped
    One score matmul per (pair, sk-tile): out[sk=128, 2*512] = krotz.T @
    qpack[:, pair-heads, q-tile] (K=128 with zero bottom half).  Score PSUM
    pairs S0/S1 double-buffer across iterations so the EXP stream on ACT never
    waits on PE.  probsT = exp(scale*s) (one ACT instr per pair-iter, fp32
    psum -> fp16); diagonal blocks masked by one 0/1-mask multiply (DVE).
    PV: one matmul out[65, 1024] = [V|1].T @ probs accumulates over sk-tiles
    (row 64 = softmax denominator).  Normalize: DVE copy of denom row,
    reciprocal_approx_fast, gpsimd partition-broadcast, DVE multiply -> a0/a1.
  phase 3  output projection, interleaved into phase 2: yo[128, 2, 512] =
    [a0|a1].T-slices @ wo in the transiently-free PV psum banks (throttled so
    a 2-tile reserve keeps the PE warm through the final normalize), cast to
    fp16 (DVE; +ACT in the drain loop), DMA'd to y; host sums the 8 per-core
    partial y in fp32.
"""

import contextlib
from collections import deque
import numpy as np
import jax.numpy as jnp

import concourse.bass as bass
import concourse.tile as tile
from concourse import bacc, mybir
from concourse.masks import make_identity
import concourse.dve_ops as _dops
from concourse.dve_spec import Spec, Src0, C0, C1


def _ref_exp_schrau(in0, in1, c0, c1, c2):
    return in0 * c0 + c1


def _register_exp_approx():
    """Schraudolph exp on the DVE: bitcast16(int16(A*x + B)) ~ exp(x*scale).
    One 2-stage custom op, 1 elem/cycle/lane -- lets the Vector engine absorb
    a slice of the softmax-exp stream that otherwise saturates ScalarE.
    Registered once per process via the documented dve_ops extension point."""
    for op in _dops.OPS:
        if op.name == "EXP_APPROX_SCHRAU":
            return op
    op = _dops.DveOp("EXP_APPROX_SCHRAU",
                     Spec(body=Src0 * C0 + C1, reference=_ref_exp_schrau),
                     subdim=False,
                     uops_sha={"v3": "2230da7084b02538", "v4": None})
    _dops.OPS.append(op)
    _dops.CUSTOM_DVE_SPECS[op.name] = op.spec
    _dops._SUB_OPCODE_FOR_NAME[op.name] = (
        _dops._CUSTOM_DVE_ROW_BASE + len(_dops.OPS) - 1)
    return op


EXP_APPROX = _register_exp_approx()
# fp16 Schraudolph constants; score scale folded into the multiplier
EXP_A16 = 1024.0 / np.log(2.0)
EXP_B16 = 15.0 * 1024.0 - 44.7

B, S, D = 2, 2048, 2048
H, KV, HD = 32, 8, 64
T = B * S
NCORES = 8
HPC = H // NCORES          # 4 query heads per core
SCALE = 1.0 / np.sqrt(HD)
THETA = 10000.0
NQT = T // 512             # 8 token tiles of 512
REPLICATED = {"xT", "cos4", "sin4", "mask"}  # same bytes on every core
NDT = D // 128             # 16 contraction tiles
F32 = mybir.dt.float32
F16 = mybir.dt.float16


def _build_program():
    nc = bacc.Bacc("TRN2", target_bir_lowering=False, debug=False)

    xT = nc.dram_tensor("xT", [D, T], F16, kind="ExternalInput")
    wq = nc.dram_tensor("wq", [D, 2 * HPC * 32], F16, kind="ExternalInput")
    wkv = nc.dram_tensor("wkv", [D, 128], F16, kind="ExternalInput")
    wo = nc.dram_tensor("wo", [HPC * HD, D], F16, kind="ExternalInput")
    cos4 = nc.dram_tensor("cos4", [128, S], F32, kind="ExternalInput")
    sin4 = nc.dram_tensor("sin4", [128, S], F32, kind="ExternalInput")
    maskd = nc.dram_tensor("mask", [128, 896], F16, kind="ExternalInput")
    y = nc.dram_tensor("y", [T, D], F16, kind="ExternalOutput")

    with tile.TileContext(nc) as tc:
        _body(tc, nc, xT, wq, wkv, wo, cos4, sin4, maskd, y)
    nc.compile()
    return nc


def _body(tc, nc, xT, wq, wkv, wo, cos4, sin4, maskd, y):
    TT = mybir.AluOpType
    # PSUM = 8 banks of [128 x 512 f32], as four 2-bank pairs:
    #   S0/S1: phase-1 qa/qb pairs; phase-2 score tiles (double-buffered).
    #   PV0/PV1: phase-1 kv + V-transpose scratch; phase-2 PV accumulators
    #            (one per head-pair); phase-3 yo tiles (in the windows where
    #            the other pair's accumulator is idle).
    ctx = contextlib.ExitStack()
    with ctx:
        const = ctx.enter_context(tc.tile_pool(name="const", bufs=1))
        persist = ctx.enter_context(tc.tile_pool(name="persist", bufs=1))
        xs = ctx.enter_context(tc.tile_pool(name="xs", bufs=6))
        rtmp = ctx.enter_context(tc.tile_pool(name="rtmp", bufs=2))
        probs = ctx.enter_context(tc.tile_pool(name="probs", bufs=3))
        norm = ctx.enter_context(tc.tile_pool(name="norm", bufs=2))
        yst = ctx.enter_context(tc.tile_pool(name="yst", bufs=3))
        psum = ctx.enter_context(tc.tile_pool(name="psum", bufs=1, space="PSUM"))

        # ---- constants ----
        # Constants go down the ACT HWDGE ring so the phase-1 x-tile stream
        # owns the SP ring exclusively.  wo is phase-3-only: last.
        wq_sbs, wkv_sbs = [], []
        wqr = wq[:, :].rearrange("(t p) c -> p t c", p=128)
        wkvr = wkv[:, :].rearrange("(t p) c -> p t c", p=128)
        for hnum in range(4):
            sl = slice(hnum * 4, (hnum + 1) * 4)
            wq_h = const.tile([128, 4, 256], F16, name=f"wq_sb{hnum}")
            nc.scalar.dma_start(out=wq_h, in_=wqr[:, sl, :])
            wq_sbs.append(wq_h)
            wkv_h = const.tile([128, 4, 128], F16, name=f"wkv_sb{hnum}")
            nc.scalar.dma_start(out=wkv_h, in_=wkvr[:, sl, :])
            wkv_sbs.append(wkv_h)
        cos_sb = const.tile([128, S], F32, name="cos_sb")
        nc.scalar.dma_start(out=cos_sb, in_=cos4[:, :])
        sin_sb = const.tile([128, S], F32, name="sin_sb")
        nc.scalar.dma_start(out=sin_sb, in_=sin4[:, :])
        # mask duplicated per head-pair half so one DVE op masks both heads
        mask2 = const.tile([128, 2, 896], F16, name="mask2")
        nc.scalar.dma_start(out=mask2[:, 0, :], in_=maskd[:, :])
        nc.scalar.dma_start(out=mask2[:, 1, :], in_=maskd[:, :])
        wo_sb = const.tile([128, 2, D], F16, name="wo_sb")
        nc.scalar.dma_start(out=wo_sb, in_=wo[:, :].rearrange("(t p) c -> p t c", p=128))
        ident = const.tile([64, 64], F16, name="ident")
        make_identity(nc, ident)

        # ---- persistent activations ----
        # qpack[(h%2)*64:(h%2)*64+64, h//2, tok] = rope'd q of head h.  The
        # score matmuls contract K=128 with zero-padded K weights (kz = [k;0],
        # zk = [0;k]) so the off-head rows annihilate; K=64 matmuls would
        # mode-switch/drain the PE (and risk the HAM clock gate).
        qpack = persist.tile([128, 2, T], F16, name="qpack")
        krotz = persist.tile([128, T], F16, name="krotz")    # rows 0:64=k', 64:128=0
        zkrot = persist.tile([128, T], F16, name="zkrot")    # rows 0:64=0, 64:128=k'
        nc.gpsimd.memset(krotz[64:128, :], 0.0)
        nc.gpsimd.memset(zkrot[0:64, :], 0.0)
        vnat = persist.tile([128, T // 128, 65], F16, name="vnat")  # [tok%128, toktile, hd+1]
        a0 = persist.tile([128, T], F16, name="a0")          # attn outT, heads 0,1
        a1 = persist.tile([128, T], F16, name="a1")          # attn outT, heads 2,3
        ones_c = const.tile([128, T // 128, 1], F16, name="ones_c")
        nc.vector.memset(ones_c, 1.0)
        nc.vector.tensor_copy(out=vnat[:, :, 64:65], in_=ones_c)

        # ================= phase 1: projections + rope =================
        SNM = ["S0", "S1"]
        PNM = ["PV0", "PV1"]
        for qt in range(NQT):
            pos0 = (qt % 4) * 512
            tok0 = qt * 512
            qab = psum.tile([128, 2, 512], F32, name=SNM[qt % 2])
            kvp = psum.tile([128, 2, 512], F32, name=PNM[qt % 2])
            kv_ps = kvp[:, 0, :]
            qa_ps = qab[:, 0, :]
            qb_ps = qab[:, 1, :]
            xt4s = []
            for d4 in range(4):
                xt4 = xs.tile([128, 4, 512], F16, name="xt4")
                nc.sync.dma_start(
                    out=xt4,
                    in_=xT[d4 * 512:(d4 + 1) * 512, tok0:tok0 + 512]
                    .rearrange("(t p) c -> p t c", p=128))
                xt4s.append(xt4)
            for d in range(NDT):
                xt = xt4s[d // 4][:, d % 4, :]
                st, sp = d == 0, d == NDT - 1
                wq_d = wq_sbs[d // 4][:, d % 4, :]
                nc.tensor.matmul(out=qa_ps, lhsT=(wq_d[:, 0:128]), rhs=(xt),
                                 start=st, stop=sp)
                nc.tensor.matmul(out=qb_ps, lhsT=(wq_d[:, 128:256]), rhs=(xt),
                                 start=st, stop=sp)
                nc.tensor.matmul(out=kv_ps, lhsT=(wkv_sbs[d // 4][:, d % 4, :]), rhs=(xt),
                                 start=st, stop=sp)
            cs = cos_sb[:, pos0:pos0 + 512]
            sn = sin_sb[:, pos0:pos0 + 512]
            # V copy first on DVE: the PE transposes wait only on it.
            vt = rtmp.tile([64, 512], F16, name="vt")
            nc.vector.tensor_copy(out=vt, in_=kvp[64:128, 0, :])
            for k4 in range(4):
                tpv = kvp[:, 1, k4 * 32:(k4 + 1) * 32].bitcast(F16)
                nc.tensor.transpose(tpv, vt[:, k4 * 128:(k4 + 1) * 128], ident)
            # K rope (single kv head): rows 0:32 ka, 32:64 kb of kv.
            k_x = rtmp.tile([32, 512], F32, name="k_x")
            k_x2 = rtmp.tile([32, 512], F32, name="k_x2")
            k_y = rtmp.tile([32, 512], F32, name="k_y")
            k_y2 = rtmp.tile([32, 512], F32, name="k_y2")
            # Q rope on [128, 512] (row 32h+r = head h dim r); both reads of
            # each psum issued back-to-back so the bank frees early.
            t_x = rtmp.tile([128, 512], F32, name="t_x")
            t_x2 = rtmp.tile([128, 512], F32, name="t_x2")
            nc.vector.tensor_tensor(out=t_x, in0=qa_ps, in1=cs, op=TT.mult)
            nc.vector.tensor_tensor(out=t_x2, in0=qa_ps, in1=sn, op=TT.mult)
            t_y = rtmp.tile([128, 512], F32, name="t_y")
            t_y2 = rtmp.tile([128, 512], F32, name="t_y2")
            nc.vector.tensor_tensor(out=t_y, in0=qb_ps, in1=sn, op=TT.mult)
            nc.vector.tensor_tensor(out=t_y2, in0=qb_ps, in1=cs, op=TT.mult)
            qra = rtmp.tile([128, 512], F16, name="qra")
            qrb = rtmp.tile([128, 512], F16, name="qrb")
            nc.vector.tensor_tensor(out=qra, in0=t_x, in1=t_y, op=TT.subtract)
            nc.vector.tensor_tensor(out=qrb, in0=t_x2, in1=t_y2, op=TT.add)
            nc.vector.tensor_tensor(out=k_x, in0=kvp[0:32, 0, :], in1=cs[0:32], op=TT.mult)
            nc.vector.tensor_tensor(out=k_x2, in0=kvp[0:32, 0, :], in1=sn[0:32], op=TT.mult)
            nc.vector.tensor_tensor(out=k_y, in0=kvp[32:64, 0, :], in1=sn[0:32], op=TT.mult)
            nc.vector.tensor_tensor(out=k_y2, in0=kvp[32:64, 0, :], in1=cs[0:32], op=TT.mult)
            # remap q into [pair-half row, pair, tok] layout for row-tiled scores
            for h in range(HPC):
                rb = (h % 2) * 64
                nc.scalar.dma_start(out=qpack[rb:rb + 32, h // 2, tok0:tok0 + 512],
                                    in_=qra[32 * h:32 * h + 32, :])
                nc.scalar.dma_start(out=qpack[rb + 32:rb + 64, h // 2, tok0:tok0 + 512],
                                    in_=qrb[32 * h:32 * h + 32, :])
            nc.vector.tensor_tensor(out=krotz[0:32, tok0:tok0 + 512], in0=k_x,
                                    in1=k_y, op=TT.subtract)
            nc.vector.tensor_tensor(out=krotz[32:64, tok0:tok0 + 512], in0=k_x2,
                                    in1=k_y2, op=TT.add)
            nc.gpsimd.tensor_copy(out=zkrot[64:128, tok0:tok0 + 512],
                                  in_=krotz[0:64, tok0:tok0 + 512])
            for k4 in range(4):
                tpv = kvp[:, 1, k4 * 32:(k4 + 1) * 32].bitcast(F16)
                nc.vector.tensor_copy(out=vnat[:, qt * 4 + k4, 0:64], in_=tpv)

        # ================= phase 2: attention (+ phase 3 interleaved) ======
        # yo tiles are emitted into the PV pair that is idle (the other
        # head-pair's accumulator), budgeted so the pair is free again before
        # the next (b, jq) needs it.
        pending = deque()   # (tt, nh) output tiles owed
        proj_ct = [0]
        drain = [False]  # in the drain loop ACT is idle: split casts across engines

        def emit_proj(pname):
            tt, nh = pending.popleft()
            yo = psum.tile([128, 2, 512], F32, name=pname)
            for half in range(2):
                n = nh * 2 + half
                nc.tensor.matmul(out=yo[:, half, :],
                                 lhsT=(a0[:, tt * 128:(tt + 1) * 128]),
                                 rhs=(wo_sb[:, 0, n * 512:(n + 1) * 512]),
                                 start=True, stop=False)
                nc.tensor.matmul(out=yo[:, half, :],
                                 lhsT=(a1[:, tt * 128:(tt + 1) * 128]),
                                 rhs=(wo_sb[:, 1, n * 512:(n + 1) * 512]),
                                 start=False, stop=True)
            stage = yst.tile([128, 2, 512], F16, name="stage")
            if proj_ct[0] % 2 == 1:
                nc.scalar.copy(out=stage, in_=yo)
            else:
                nc.vector.tensor_copy(out=stage, in_=yo)
            proj_ct[0] += 1
            nc.sync.dma_start(out=y[tt * 128:(tt + 1) * 128,
                                    nh * 1024:(nh + 1) * 1024], in_=stage)

        si = 0
        unit = 0
        for b in range(B):
            for jq in range(4):
                tq = b * S + jq * 512
                ni = 4 * jq + 4
                for pair in range(2):
                    pvp = psum.tile([65, 2, 512], F32, name=PNM[pair])
                    pend = None  # probs tile not yet fed to PV
                    for i in range(ni):
                        tk = b * S + i * 128
                        sc = psum.tile([128, 2, 512], F32, name=SNM[si % 2])
                        si += 1
                        for h2, kt in ((0, krotz), (1, zkrot)):
                            nc.tensor.matmul(
                                out=sc[:, h2, :], lhsT=(kt[:, tk:tk + 128]),
                                rhs=(qpack[:, pair, tq:tq + 512]),
                                start=True, stop=True)
                        if pend is not None:
                            ip, ptp = pend
                            for h2 in range(2):
                                nc.tensor.matmul(out=pvp[:, h2, :],
                                                 lhsT=(vnat[:, b * 16 + ip, :]),
                                                 rhs=(ptp[:, h2, :]),
                                                 start=ip == 0, stop=False)
                        pt = probs.tile([128, 2, 512], F16, name="pt")
                        nc.scalar.activation(out=pt, in_=sc,
                                             func=mybir.ActivationFunctionType.Exp,
                                             scale=float(SCALE))
                        unit += 1
                        if i >= 4 * jq:  # diagonal block: causal mask
                            roff = 128 * i - 512 * jq
                            nc.vector.tensor_tensor(
                                out=pt, in0=pt,
                                in1=mask2[:, :, 384 - roff:896 - roff], op=TT.mult)
                        pend = (i, pt)
                        # proj tiles go out after the mask so their psum cast
                        # queues BEHIND it on the DVE (a cast ahead of the mask
                        # head-of-line blocks the PV matmul and stalls the PE);
                        # ni=4 windows are shorter than the yo bank cycle: skip.
                        if len(pending) > 2 and i % 2 == 1 and ni >= 8:
                            emit_proj(PNM[1 - pair])
                    ip, ptp = pend
                    for h2 in range(2):
                        nc.tensor.matmul(out=pvp[:, h2, :],
                                         lhsT=(vnat[:, b * 16 + ip, :]),
                                         rhs=(ptp[:, h2, :]),
                                         start=ip == 0, stop=True)
                    # normalize: row 64 of pvp is the softmax denominator.
                    # (the custom-DVE reciprocal reads garbage from PSUM on
                    # HW -- bounce the denominator row through SBUF first.)
                    sums = norm.tile([1, 2, 512], F32, name="sums")
                    nc.vector.tensor_copy(out=sums, in_=pvp[64:65, :, :])
                    rec = norm.tile([1, 2, 512], F32, name="rec")
                    nc.vector.reciprocal_approx_fast(out=rec, in_=sums)
                    dst = a0 if pair == 0 else a1
                    for h2 in range(2):
                        rbc = norm.tile([64, 512], F32, name="rbc")
                        nc.gpsimd.partition_broadcast(rbc, rec[0:1, h2, :])
                        nc.vector.tensor_tensor(out=dst[h2 * 64:h2 * 64 + 64, tq:tq + 512],
                                                in0=pvp[0:64, h2, :], in1=rbc,
                                                op=TT.mult)
                # output tiles of this (b, jq) are complete after both pairs
                tt0 = b * 16 + jq * 4
                pending.extend((tt0 + t, nh) for t in range(4) for nh in range(2))

        # remaining projections rotate through all four freed psum pairs
        drain[0] = True
        k = 0
        while pending:
            emit_proj((SNM + PNM)[k % 4])
            k += 1


_CACHE = {}


def _get_program():
    if "nc" not in _CACHE:
        _CACHE["nc"] = _build_program()
    return _CACHE["nc"]


def _get_runner():
    """Cached jitted shard_map executable over 8 cores (avoids per-call
    retrace that run_bass_kernel_spmd pays)."""
    if "runner" in _CACHE:
        return _CACHE["runner"]
    import jax
    from jax.sharding import Mesh, PartitionSpec
    from jax.experimental.shard_map import shard_map
    from concourse import bass2jax
    from concourse.bass2jax import _bass_exec_p

    bass2jax.install_neuronx_cc_hook()
    nc = _get_program()
    partition_name = nc.partition_id_tensor.name if nc.partition_id_tensor else None
    in_names, out_names, out_avals = [], [], []
    for alloc in nc.m.functions[0].allocations:
        if not isinstance(alloc, mybir.MemoryLocationSet):
            continue
        name = alloc.memorylocations[0].name
        if alloc.kind == "ExternalInput":
            if name != partition_name:
                in_names.append(name)
        elif alloc.kind == "ExternalOutput":
            out_names.append(name)
            out_avals.append(jax.core.ShapedArray(
                tuple(alloc.tensor_shape), mybir.dt.np(alloc.dtype)))
    n_params = len(in_names)
    n_outs = len(out_avals)
    all_in = list(in_names) + list(out_names)
    if partition_name is not None:
        all_in.append(partition_name)

    def _body_fn(*args):
        operands = list(args)
        if partition_name is not None:
            operands.append(bass2jax.partition_id_tensor())
        return tuple(_bass_exec_p.bind(
            *operands,
            out_avals=tuple(out_avals),
            in_names=tuple(all_in),
            out_names=tuple(out_names),
            lowering_input_output_aliases=(),
            sim_require_finite=True,
            sim_require_nnan=True,
            nc=nc,
        ))

    devices = jax.devices()[:NCORES]
    mesh = Mesh(np.asarray(devices), ("core",))
    # xT / rope tables / mask are identical on every core: feed them
    # replicated (P()) so the host uploads one copy + on-device all-gather,
    # instead of 8 copies through the tunnel.
    in_specs = tuple(
        PartitionSpec() if n in REPLICATED else PartitionSpec("core")
        for n in in_names) + (PartitionSpec("core"),) * n_outs
    sharded = jax.jit(
        shard_map(_body_fn, mesh=mesh,
                  in_specs=in_specs,
                  out_specs=(PartitionSpec("core"),) * n_outs,
                  check_rep=False),
        donate_argnums=tuple(range(n_params, n_params + n_outs)),
        keep_unused=True)

    from jax.sharding import NamedSharding
    rep = NamedSharding(mesh, PartitionSpec())
    shd = NamedSharding(mesh, PartitionSpec("core"))
    gather = jax.jit(lambda a: a, out_shardings=rep)   # upload-shard -> all-gather
    zeros = jax.jit(lambda: jnp.zeros((NCORES * T, D), jnp.float16),
                    out_shardings=shd)
    reduce_y = jax.jit(lambda yc: yc.reshape(NCORES, T, D)
                       .sum(0, dtype=jnp.float32), out_shardings=rep)
    _CACHE["runner"] = (sharded, in_names, out_names, out_avals,
                        mesh, rep, shd, gather, zeros, reduce_y)
    return _CACHE["runner"]


def _host_inputs(x, wq, wk, wv, wo):
    x = np.asarray(x, np.float32)
    wq = np.asarray(wq, np.float16)
    wk = np.asarray(wk, np.float16)
    wv = np.asarray(wv, np.float16)
    wo = np.asarray(wo, np.float16)

    xT = np.ascontiguousarray(x.reshape(T, D).T.astype(np.float16))

    inv = 1.0 / (THETA ** (np.arange(0, HD, 2, dtype=np.float64) / HD))
    fr = np.outer(inv, np.arange(S, dtype=np.float64))   # [32, S]
    cosT = np.cos(fr).astype(np.float32)
    sinT = np.sin(fr).astype(np.float32)
    cos4 = np.ascontiguousarray(np.tile(cosT, (4, 1)))
    sin4 = np.ascontiguousarray(np.tile(sinT, (4, 1)))

    u = np.arange(896)[None, :]
    p = np.arange(128)[:, None]
    mask = (u >= p + 384).astype(np.float16)

    in_maps = []
    for c in range(NCORES):
        cols_a, cols_b = [], []
        for h in range(HPC):
            base = (HPC * c + h) * HD
            cols_a.append(wq[:, base:base + 32])
            cols_b.append(wq[:, base + 32:base + 64])
        wq_c = np.ascontiguousarray(np.concatenate(cols_a + cols_b, axis=1))
        kb = c * HD
        wkv_c = np.ascontiguousarray(np.concatenate(
            [wk[:, kb:kb + 32], wk[:, kb + 32:kb + 64], wv[:, kb:kb + HD]], axis=1))
        wo_c = np.ascontiguousarray(wo[c * HPC * HD:(c + 1) * HPC * HD, :])
        in_maps.append({"xT": xT, "wq": wq_c, "wkv": wkv_c, "wo": wo_c,
                        "cos4": cos4, "sin4": sin4, "mask": mask})
    return in_maps


def _stage_inputs(in_maps):
    """Upload inputs: replicated tensors go up as 1/8 shards and are
    all-gathered on device; per-core tensors upload as the usual concat."""
    import jax
    (sharded, in_names, out_names, out_avals,
     mesh, rep, shd, gather, zeros, reduce_y) = _get_runner()
    staged = []
    for n in in_names:
        if n in REPLICATED:
            a = in_maps[0][n]
            if a.shape[0] % NCORES == 0:
                staged.append(gather(jax.device_put(a, shd)))
            else:
                staged.append(jax.device_put(a, rep))
        else:
            cat = np.concatenate([m[n] for m in in_maps], axis=0)
            staged.append(jax.device_put(cat, shd))
    return staged


def kernel(x, wq, wk, wv, wo):
    import jax
    (sharded, in_names, out_names, out_avals,
     mesh, rep, shd, gather, zeros, reduce_y) = _get_runner()
    in_maps = _host_inputs(x, wq, wk, wv, wo)
    staged = _stage_inputs(in_maps)
    out_arrs = sharded(*staged, zeros())
    ysum = reduce_y(out_arrs[out_names.index("y")])
    return np.asarray(ysum).reshape(B, S, D)


# revision 13
# speedup vs baseline: 1.0075x; 1.0075x over previous
"""Grouped-Query Attention (B=2, S=2048, D=2048, H=32, KV=8, HD=64) on 8 TRN2
NeuronCores, tensor-parallel over KV-head groups (1 KV head + 4 Q heads per
core), with host-side shard/gather.

All matmul operands are fp16 (PE streams 1 cyc/row; PSUM accumulation fp32),
elementwise RoPE/softmax math fp32 where it reads PSUM.

Per-core dataflow (activations kept feature-on-partitions so every matmul
contracts over the partition dim with no on-device transposition of x):

  phase 1  QKV projection + RoPE
    xT d-tiles stream in batches of 4 (one DMA each) -> psum: qa/qb (S pair),
    kv (PV pair); RoPE on DVE from PSUM; Q written to qpack[0:64, h, tok]
    (rows 64:128 stay zero -- they face zero K-weights in the score matmul);
    K written to krotz rows 0:64 (rows 64:128 zero); V transposed back to
    natural [tok, hd] via PE transpose, + a ones column (softmax denominator).
  phase 2  attention per (batch, q-tile 512, head-pair), causal-block-skipped
    One score matmul per (pair, sk-tile): out[sk=128, 2*512] = krotz.T @
    qpack[:, pair-heads, q-tile] (K=128 with zero bottom half).  Score PSUM
    pairs S0/S1 double-buffer across iterations so the EXP stream on ACT never
    waits on PE.  probsT = exp(scale*s) (one ACT instr per pair-iter, fp32
    psum -> fp16); diagonal blocks masked by one 0/1-mask multiply (DVE).
    PV: one matmul out[65, 1024] = [V|1].T @ probs accumulates over sk-tiles
    (row 64 = softmax denominator).  Normalize: DVE copy of denom row,
    reciprocal_approx_fast, gpsimd partition-broadcast, DVE multiply -> a0/a1.
  phase 3  output projection, interleaved into phase 2: yo[128, 2, 512] =
    [a0|a1].T-slices @ wo in the transiently-free PV psum banks (throttled so
    a 2-tile reserve keeps the PE warm through the final normalize), cast to
    fp16 (DVE; +ACT in the drain loop), DMA'd to y; host sums the 8 per-core
    partial y in fp32.
"""

import contextlib
from collections import deque
import numpy as np
import jax.numpy as jnp

import concourse.bass as bass
import concourse.tile as tile
from concourse import bacc, mybir
from concourse.masks import make_identity
import concourse.dve_ops as _dops
from concourse.dve_spec import Spec, Src0, C0, C1


def _ref_exp_schrau(in0, in1, c0, c1, c2):
    return in0 * c0 + c1


def _register_exp_approx():
    """Schraudolph exp on the DVE: bitcast16(int16(A*x + B)) ~ exp(x*scale).
    One 2-stage custom op, 1 elem/cycle/lane -- lets the Vector engine absorb
    a slice of the softmax-exp stream that otherwise saturates ScalarE.
    Registered once per process via the documented dve_ops extension point."""
    for op in _dops.OPS:
        if op.name == "EXP_APPROX_SCHRAU":
            return op
    op = _dops.DveOp("EXP_APPROX_SCHRAU",
                     Spec(body=Src0 * C0 + C1, reference=_ref_exp_schrau),
                     subdim=False,
                     uops_sha={"v3": "2230da7084b02538", "v4": None})
    _dops.OPS.append(op)
    _dops.CUSTOM_DVE_SPECS[op.name] = op.spec
    _dops._SUB_OPCODE_FOR_NAME[op.name] = (
        _dops._CUSTOM_DVE_ROW_BASE + len(_dops.OPS) - 1)
    return op


EXP_APPROX = _register_exp_approx()
# fp16 Schraudolph constants; score scale folded into the multiplier
EXP_A16 = 1024.0 / np.log(2.0)
EXP_B16 = 15.0 * 1024.0 - 44.7

B, S, D = 2, 2048, 2048
H, KV, HD = 32, 8, 64
T = B * S
NCORES = 8
HPC = H // NCORES          # 4 query heads per core
SCALE = 1.0 / np.sqrt(HD)
THETA = 10000.0
NQT = T // 512             # 8 token tiles of 512
REPLICATED = {"xT", "cos4", "sin4", "mask"}  # same bytes on every core
NDT = D // 128             # 16 contraction tiles
F32 = mybir.dt.float32
F16 = mybir.dt.float16


def _build_program():
    nc = bacc.Bacc("TRN2", target_bir_lowering=False, debug=False)

    xT = nc.dram_tensor("xT", [D, T], F16, kind="ExternalInput")
    wq = nc.dram_tensor("wq", [D, 2 * HPC * 32], F16, kind="ExternalInput")
    wkv = nc.dram_tensor("wkv", [D, 128], F16, kind="ExternalInput")
    wo = nc.dram_tensor("wo", [HPC * HD, D], F16, kind="ExternalInput")
    cos4 = nc.dram_tensor("cos4", [128, S], F32, kind="ExternalInput")
    sin4 = nc.dram_tensor("sin4", [128, S], F32, kind="ExternalInput")
    maskd = nc.dram_tensor("mask", [128, 896], F16, kind="ExternalInput")
    y = nc.dram_tensor("y", [T, D], F16, kind="ExternalOutput")

    with tile.TileContext(nc) as tc:
        _body(tc, nc, xT, wq, wkv, wo, cos4, sin4, maskd, y)
    nc.compile()
    return nc


def _body(tc, nc, xT, wq, wkv, wo, cos4, sin4, maskd, y):
    TT = mybir.AluOpType
    # PSUM = 8 banks of [128 x 512 f32], as four 2-bank pairs:
    #   S0/S1: phase-1 qa/qb pairs; phase-2 score tiles (double-buffered).
    #   PV0/PV1: phase-1 kv + V-transpose scratch; phase-2 PV accumulators
    #            (one per head-pair); phase-3 yo tiles (in the windows where
    #            the other pair's accumulator is idle).
    ctx = contextlib.ExitStack()
    with ctx:
        const = ctx.enter_context(tc.tile_pool(name="const", bufs=1))
        persist = ctx.enter_context(tc.tile_pool(name="persist", bufs=1))
        xs = ctx.enter_context(tc.tile_pool(name="xs", bufs=6))
        rtmp = ctx.enter_context(tc.tile_pool(name="rtmp", bufs=2))
        probs = ctx.enter_context(tc.tile_pool(name="probs", bufs=3))
        norm = ctx.enter_context(tc.tile_pool(name="norm", bufs=2))
        yst = ctx.enter_context(tc.tile_pool(name="yst", bufs=3))
        psum = ctx.enter_context(tc.tile_pool(name="psum", bufs=1, space="PSUM"))

        # ---- constants ----
        # Constants go down the ACT HWDGE ring so the phase-1 x-tile stream
        # owns the SP ring exclusively.  wo is phase-3-only: last.
        wq_sbs, wkv_sbs = [], []
        wqr = wq[:, :].rearrange("(t p) c -> p t c", p=128)
        wkvr = wkv[:, :].rearrange("(t p) c -> p t c", p=128)
        for hnum in range(4):
            sl = slice(hnum * 4, (hnum + 1) * 4)
            wq_h = const.tile([128, 4, 256], F16, name=f"wq_sb{hnum}")
            nc.scalar.dma_start(out=wq_h, in_=wqr[:, sl, :])
            wq_sbs.append(wq_h)
            wkv_h = const.tile([128, 4, 128], F16, name=f"wkv_sb{hnum}")
            nc.scalar.dma_start(out=wkv_h, in_=wkvr[:, sl, :])
            wkv_sbs.append(wkv_h)
        cos_sb = const.tile([128, S], F32, name="cos_sb")
        nc.scalar.dma_start(out=cos_sb, in_=cos4[:, :])
        sin_sb = const.tile([128, S], F32, name="sin_sb")
        nc.scalar.dma_start(out=sin_sb, in_=sin4[:, :])
        # mask duplicated per head-pair half so one DVE op masks both heads
        mask2 = const.tile([128, 2, 896], F16, name="mask2")
        nc.scalar.dma_start(out=mask2[:, 0, :], in_=maskd[:, :])
        nc.scalar.dma_start(out=mask2[:, 1, :], in_=maskd[:, :])
        wo_sb = const.tile([128, 2, D], F16, name="wo_sb")
        nc.scalar.dma_start(out=wo_sb, in_=wo[:, :].rearrange("(t p) c -> p t c", p=128))
        ident = const.tile([64, 64], F16, name="ident")
        make_identity(nc, ident)

        # ---- persistent activations ----
        # qpack[(h%2)*64:(h%2)*64+64, h//2, tok] = rope'd q of head h.  The
        # score matmuls contract K=128 with zero-padded K weights (kz = [k;0],
        # zk = [0;k]) so the off-head rows annihilate; K=64 matmuls would
        # mode-switch/drain the PE (and risk the HAM clock gate).
        qpack = persist.tile([128, 2, T], F16, name="qpack")
        krotz = persist.tile([128, T], F16, name="krotz")    # rows 0:64=k', 64:128=0
        zkrot = persist.tile([128, T], F16, name="zkrot")    # rows 0:64=0, 64:128=k'
        nc.gpsimd.memset(krotz[64:128, :], 0.0)
        nc.gpsimd.memset(zkrot[0:64, :], 0.0)
        vnat = persist.tile([128, T // 128, 65], F16, name="vnat")  # [tok%128, toktile, hd+1]
        a0 = persist.tile([128, T], F16, name="a0")          # attn outT, heads 0,1
        a1 = persist.tile([128, T], F16, name="a1")          # attn outT, heads 2,3
        ones_c = const.tile([128, T // 128, 1], F16, name="ones_c")
        nc.vector.memset(ones_c, 1.0)
        nc.vector.tensor_copy(out=vnat[:, :, 64:65], in_=ones_c)

        # ================= phase 1: projections + rope =================
        SNM = ["S0", "S1"]
        PNM = ["PV0", "PV1"]
        for qt in range(NQT):
            pos0 = (qt % 4) * 512
            tok0 = qt * 512
            qab = psum.tile([128, 2, 512], F32, name=SNM[qt % 2])
            kvp = psum.tile([128, 2, 512], F32, name=PNM[qt % 2])
            kv_ps = kvp[:, 0, :]
            qa_ps = qab[:, 0, :]
            qb_ps = qab[:, 1, :]
            xt4s = []
            for d4 in range(4):
                xt4 = xs.tile([128, 4, 512], F16, name="xt4")
                nc.sync.dma_start(
                    out=xt4,
                    in_=xT[d4 * 512:(d4 + 1) * 512, tok0:tok0 + 512]
                    .rearrange("(t p) c -> p t c", p=128))
                xt4s.append(xt4)
            for d in range(NDT):
                xt = xt4s[d // 4][:, d % 4, :]
                st, sp = d == 0, d == NDT - 1
                wq_d = wq_sbs[d // 4][:, d % 4, :]
                nc.tensor.matmul(out=qa_ps, lhsT=(wq_d[:, 0:128]), rhs=(xt),
                                 start=st, stop=sp)
                nc.tensor.matmul(out=qb_ps, lhsT=(wq_d[:, 128:256]), rhs=(xt),
                                 start=st, stop=sp)
                nc.tensor.matmul(out=kv_ps, lhsT=(wkv_sbs[d // 4][:, d % 4, :]), rhs=(xt),
                                 start=st, stop=sp)
            cs = cos_sb[:, pos0:pos0 + 512]
            sn = sin_sb[:, pos0:pos0 + 512]
            # V copy first on DVE: the PE transposes wait only on it.
            vt = rtmp.tile([64, 512], F16, name="vt")
            nc.vector.tensor_copy(out=vt, in_=kvp[64:128, 0, :])
            for k4 in range(4):
                tpv = kvp[:, 1, k4 * 32:(k4 + 1) * 32].bitcast(F16)
                nc.tensor.transpose(tpv, vt[:, k4 * 128:(k4 + 1) * 128], ident)
            # K rope (single kv head): rows 0:32 ka, 32:64 kb of kv.
            k_x = rtmp.tile([32, 512], F32, name="k_x")
            k_x2 = rtmp.tile([32, 512], F32, name="k_x2")
            k_y = rtmp.tile([32, 512], F32, name="k_y")
            k_y2 = rtmp.tile([32, 512], F32, name="k_y2")
            # Q rope on [128, 512] (row 32h+r = head h dim r); both reads of
            # each psum issued back-to-back so the bank frees early.
            t_x = rtmp.tile([128, 512], F32, name="t_x")
            t_x2 = rtmp.tile([128, 512], F32, name="t_x2")
            nc.vector.tensor_tensor(out=t_x, in0=qa_ps, in1=cs, op=TT.mult)
            nc.vector.tensor_tensor(out=t_x2, in0=qa_ps, in1=sn, op=TT.mult)
            t_y = rtmp.tile([128, 512], F32, name="t_y")
            t_y2 = rtmp.tile([128, 512], F32, name="t_y2")
            nc.vector.tensor_tensor(out=t_y, in0=qb_ps, in1=sn, op=TT.mult)
            nc.vector.tensor_tensor(out=t_y2, in0=qb_ps, in1=cs, op=TT.mult)
            qra = rtmp.tile([128, 512], F16, name="qra")
            qrb = rtmp.tile([128, 512], F16, name="qrb")
            nc.vector.tensor_tensor(out=qra, in0=t_x, in1=t_y, op=TT.subtract)
            nc.vector.tensor_tensor(out=qrb, in0=t_x2, in1=t_y2, op=TT.add)
            nc.vector.tensor_tensor(out=k_x, in0=kvp[0:32, 0, :], in1=cs[0:32], op=TT.mult)
            nc.vector.tensor_tensor(out=k_x2, in0=kvp[0:32, 0, :], in1=sn[0:32], op=TT.mult)
            nc.vector.tensor_tensor(out=k_y, in0=kvp[32:64, 0, :], in1=sn[0:32], op=TT.mult)
            nc.vector.tensor_tensor(out=k_y2, in0=kvp[32:64, 0, :], in1=cs[0:32], op=TT.mult)
            # remap q into [pair-half row, pair, tok] layout for row-tiled scores
            for h in range(HPC):
                rb = (h % 2) * 64
                nc.scalar.dma_start(out=qpack[rb:rb + 32, h // 2, tok0:tok0 + 512],
                                    in_=qra[32 * h:32 * h + 32, :])
                nc.scalar.dma_start(out=qpack[rb + 32:rb + 64, h // 2, tok0:tok0 + 512],
                                    in_=qrb[32 * h:32 * h + 32, :])
            nc.vector.tensor_tensor(out=krotz[0:32, tok0:tok0 + 512], in0=k_x,
                                    in1=k_y, op=TT.subtract)
            nc.vector.tensor_tensor(out=krotz[32:64, tok0:tok0 + 512], in0=k_x2,
                                    in1=k_y2, op=TT.add)
            nc.gpsimd.tensor_copy(out=zkrot[64:128, tok0:tok0 + 512],
                                  in_=krotz[0:64, tok0:tok0 + 512])
            for k4 in range(4):
                tpv = kvp[:, 1, k4 * 32:(k4 + 1) * 32].bitcast(F16)
                nc.vector.tensor_copy(out=vnat[:, qt * 4 + k4, 0:64], in_=tpv)

        # ================= phase 2: attention (+ phase 3 interleaved) ======
        # yo tiles are emitted into the PV pair that is idle (the other
        # head-pair's accumulator), budgeted so the pair is free again before
        # the next (b, jq) needs it.
        pending = deque()   # (tt, nh) output tiles owed
        proj_ct = [0]
        drain = [False]  # in the drain loop ACT is idle: split casts across engines

        def emit_proj(pname):
            tt, nh = pending.popleft()
            yo = psum.tile([128, 2, 512], F32, name=pname)
            for half in range(2):
                n = nh * 2 + half
                nc.tensor.matmul(out=yo[:, half, :],
                                 lhsT=(a0[:, tt * 128:(tt + 1) * 128]),
                                 rhs=(wo_sb[:, 0, n * 512:(n + 1) * 512]),
                                 start=True, stop=False)
                nc.tensor.matmul(out=yo[:, half, :],
                                 lhsT=(a1[:, tt * 128:(tt + 1) * 128]),
                                 rhs=(wo_sb[:, 1, n * 512:(n + 1) * 512]),
                                 start=False, stop=True)
            stage = yst.tile([128, 2, 512], F16, name="stage")
            # in-loop casts stay on the DVE: a copy on ScalarE would queue
            # ahead of the next EXP in its FIFO and stall the score pipeline
            if drain[0] and proj_ct[0] % 2 == 1:
                nc.scalar.copy(out=stage, in_=yo)
            else:
                nc.vector.tensor_copy(out=stage, in_=yo)
            proj_ct[0] += 1
            nc.sync.dma_start(out=y[tt * 128:(tt + 1) * 128,
                                    nh * 1024:(nh + 1) * 1024], in_=stage)

        si = 0
        unit = 0
        for b in range(B):
            for jq in range(4):
                tq = b * S + jq * 512
                ni = 4 * jq + 4
                for pair in range(2):
                    pvp = psum.tile([65, 2, 512], F32, name=PNM[pair])
                    pend = None  # probs tile not yet fed to PV
                    for i in range(ni):
                        tk = b * S + i * 128
                        sc = psum.tile([128, 2, 512], F32, name=SNM[si % 2])
                        si += 1
                        for h2, kt in ((0, krotz), (1, zkrot)):
                            nc.tensor.matmul(
                                out=sc[:, h2, :], lhsT=(kt[:, tk:tk + 128]),
                                rhs=(qpack[:, pair, tq:tq + 512]),
                                start=True, stop=True)
                        if pend is not None:
                            ip, ptp = pend
                            for h2 in range(2):
                                nc.tensor.matmul(out=pvp[:, h2, :],
                                                 lhsT=(vnat[:, b * 16 + ip, :]),
                                                 rhs=(ptp[:, h2, :]),
                                                 start=ip == 0, stop=False)
                        pt = probs.tile([128, 2, 512], F16, name="pt")
                        nc.scalar.activation(out=pt, in_=sc,
                                             func=mybir.ActivationFunctionType.Exp,
                                             scale=float(SCALE))
                        unit += 1
                        if i >= 4 * jq:  # diagonal block: causal mask
                            roff = 128 * i - 512 * jq
                            nc.vector.tensor_tensor(
                                out=pt, in0=pt,
                                in1=mask2[:, :, 384 - roff:896 - roff], op=TT.mult)
                        pend = (i, pt)
                        # proj tiles go out after the mask so their psum cast
                        # queues BEHIND it on the DVE (a cast ahead of the mask
                        # head-of-line blocks the PV matmul and stalls the PE);
                        # ni=4 windows are shorter than the yo bank cycle: skip.
                        if len(pending) > 2 and i % 2 == 1 and ni >= 8:
                            emit_proj(PNM[1 - pair])
                    ip, ptp = pend
                    for h2 in range(2):
                        nc.tensor.matmul(out=pvp[:, h2, :],
                                         lhsT=(vnat[:, b * 16 + ip, :]),
                                         rhs=(ptp[:, h2, :]),
                                         start=ip == 0, stop=True)
                    # normalize: row 64 of pvp is the softmax denominator.
                    # (the custom-DVE reciprocal reads garbage from PSUM on
                    # HW -- bounce the denominator row through SBUF first.)
                    sums = norm.tile([1, 2, 512], F32, name="sums")
                    nc.vector.tensor_copy(out=sums, in_=pvp[64:65, :, :])
                    rec = norm.tile([1, 2, 512], F32, name="rec")
                    nc.vector.reciprocal_approx_fast(out=rec, in_=sums)
                    dst = a0 if pair == 0 else a1
                    for h2 in range(2):
                        rbc = norm.tile([64, 512], F32, name="rbc")
                        nc.gpsimd.partition_broadcast(rbc, rec[0:1, h2, :])
                        nc.vector.tensor_tensor(out=dst[h2 * 64:h2 * 64 + 64, tq:tq + 512],
                                                in0=pvp[0:64, h2, :], in1=rbc,
                                                op=TT.mult)
                # output tiles of this (b, jq) are complete after both pairs
                tt0 = b * 16 + jq * 4
                pending.extend((tt0 + t, nh) for t in range(4) for nh in range(2))

        # remaining projections rotate through all four freed psum pairs
        drain[0] = True
        k = 0
        while pending:
            emit_proj((SNM + PNM)[k % 4])
            k += 1


_CACHE = {}


def _get_program():
    if "nc" not in _CACHE:
        _CACHE["nc"] = _build_program()
    return _CACHE["nc"]


def _get_runner():
    """Cached jitted shard_map executable over 8 cores (avoids per-call
    retrace that run_bass_kernel_spmd pays)."""
    if "runner" in _CACHE:
        return _CACHE["runner"]
    import jax
    from jax.sharding import Mesh, PartitionSpec
    from jax.experimental.shard_map import shard_map
    from concourse import bass2jax
    from concourse.bass2jax import _bass_exec_p

    bass2jax.install_neuronx_cc_hook()
    nc = _get_program()
    partition_name = nc.partition_id_tensor.name if nc.partition_id_tensor else None
    in_names, out_names, out_avals = [], [], []
    for alloc in nc.m.functions[0].allocations:
        if not isinstance(alloc, mybir.MemoryLocationSet):
            continue
        name = alloc.memorylocations[0].name
        if alloc.kind == "ExternalInput":
            if name != partition_name:
                in_names.append(name)
        elif alloc.kind == "ExternalOutput":
            out_names.append(name)
            out_avals.append(jax.core.ShapedArray(
                tuple(alloc.tensor_shape), mybir.dt.np(alloc.dtype)))
    n_params = len(in_names)
    n_outs = len(out_avals)
    all_in = list(in_names) + list(out_names)
    if partition_name is not None:
        all_in.append(partition_name)

    def _body_fn(*args):
        operands = list(args)
        if partition_name is not None:
            operands.append(bass2jax.partition_id_tensor())
        return tuple(_bass_exec_p.bind(
            *operands,
            out_avals=tuple(out_avals),
            in_names=tuple(all_in),
            out_names=tuple(out_names),
            lowering_input_output_aliases=(),
            sim_require_finite=True,
            sim_require_nnan=True,
            nc=nc,
        ))

    devices = jax.devices()[:NCORES]
    mesh = Mesh(np.asarray(devices), ("core",))
    # xT / rope tables / mask are identical on every core: feed them
    # replicated (P()) so the host uploads one copy + on-device all-gather,
    # instead of 8 copies through the tunnel.
    in_specs = tuple(
        PartitionSpec() if n in REPLICATED else PartitionSpec("core")
        for n in in_names) + (PartitionSpec("core"),) * n_outs
    sharded = jax.jit(
        shard_map(_body_fn, mesh=mesh,
                  in_specs=in_specs,
                  out_specs=(PartitionSpec("core"),) * n_outs,
                  check_rep=False),
        donate_argnums=tuple(range(n_params, n_params + n_outs)),
        keep_unused=True)

    from jax.sharding import NamedSharding
    rep = NamedSharding(mesh, PartitionSpec())
    shd = NamedSharding(mesh, PartitionSpec("core"))
    gather = jax.jit(lambda a: a, out_shardings=rep)   # upload-shard -> all-gather
    zeros = jax.jit(lambda: jnp.zeros((NCORES * T, D), jnp.float16),
                    out_shardings=shd)
    reduce_y = jax.jit(lambda yc: yc.reshape(NCORES, T, D)
                       .sum(0, dtype=jnp.float32), out_shardings=rep)
    _CACHE["runner"] = (sharded, in_names, out_names, out_avals,
                        mesh, rep, shd, gather, zeros, reduce_y)
    return _CACHE["runner"]


def _host_inputs(x, wq, wk, wv, wo):
    x = np.asarray(x, np.float32)
    wq = np.asarray(wq, np.float16)
    wk = np.asarray(wk, np.float16)
    wv = np.asarray(wv, np.float16)
    wo = np.asarray(wo, np.float16)

    xT = np.ascontiguousarray(x.reshape(T, D).T.astype(np.float16))

    inv = 1.0 / (THETA ** (np.arange(0, HD, 2, dtype=np.float64) / HD))
    fr = np.outer(inv, np.arange(S, dtype=np.float64))   # [32, S]
    cosT = np.cos(fr).astype(np.float32)
    sinT = np.sin(fr).astype(np.float32)
    cos4 = np.ascontiguousarray(np.tile(cosT, (4, 1)))
    sin4 = np.ascontiguousarray(np.tile(sinT, (4, 1)))

    u = np.arange(896)[None, :]
    p = np.arange(128)[:, None]
    mask = (u >= p + 384).astype(np.float16)

    in_maps = []
    for c in range(NCORES):
        cols_a, cols_b = [], []
        for h in range(HPC):
            base = (HPC * c + h) * HD
            cols_a.append(wq[:, base:base + 32])
            cols_b.append(wq[:, base + 32:base + 64])
        wq_c = np.ascontiguousarray(np.concatenate(cols_a + cols_b, axis=1))
        kb = c * HD
        wkv_c = np.ascontiguousarray(np.concatenate(
            [wk[:, kb:kb + 32], wk[:, kb + 32:kb + 64], wv[:, kb:kb + HD]], axis=1))
        wo_c = np.ascontiguousarray(wo[c * HPC * HD:(c + 1) * HPC * HD, :])
        in_maps.append({"xT": xT, "wq": wq_c, "wkv": wkv_c, "wo": wo_c,
                        "cos4": cos4, "sin4": sin4, "mask": mask})
    return in_maps


def _stage_inputs(in_maps):
    """Upload inputs: replicated tensors go up as 1/8 shards and are
    all-gathered on device; per-core tensors upload as the usual concat."""
    import jax
    (sharded, in_names, out_names, out_avals,
     mesh, rep, shd, gather, zeros, reduce_y) = _get_runner()
    staged = []
    for n in in_names:
        if n in REPLICATED:
            a = in_maps[0][n]
            if a.shape[0] % NCORES == 0:
                staged.append(gather(jax.device_put(a, shd)))
            else:
                staged.append(jax.device_put(a, rep))
        else:
            cat = np.concatenate([m[n] for m in in_maps], axis=0)
            staged.append(jax.device_put(cat, shd))
    return staged


def kernel(x, wq, wk, wv, wo):
    import jax
    (sharded, in_names, out_names, out_avals,
     mesh, rep, shd, gather, zeros, reduce_y) = _get_runner()
    in_maps = _host_inputs(x, wq, wk, wv, wo)
    staged = _stage_inputs(in_maps)
    out_arrs = sharded(*staged, zeros())
    ysum = reduce_y(out_arrs[out_names.index("y")])
    return np.asarray(ysum).reshape(B, S, D)


# revision 14
# speedup vs baseline: 1.0230x; 1.0153x over previous
"""Grouped-Query Attention (B=2, S=2048, D=2048, H=32, KV=8, HD=64) on 8 TRN2
NeuronCores, tensor-parallel over KV-head groups (1 KV head + 4 Q heads per
core), with host-side shard/gather.

All matmul operands are fp16 (PE streams 1 cyc/row; PSUM accumulation fp32),
elementwise RoPE/softmax math fp32 where it reads PSUM.

Per-core dataflow (activations kept feature-on-partitions so every matmul
contracts over the partition dim with no on-device transposition of x):

  phase 1  QKV projection + RoPE
    xT d-tiles stream in batches of 4 (one DMA each) -> psum: qa/qb (S pair),
    kv (PV pair); RoPE on DVE from PSUM; Q written to qpack[0:64, h, tok]
    (rows 64:128 stay zero -- they face zero K-weights in the score matmul);
    K written to krotz rows 0:64 (rows 64:128 zero); V transposed back to
    natural [tok, hd] via PE transpose, + a ones column (softmax denominator).
  phase 2  attention per (batch, q-tile 512, head-pair), causal-block-skipped
    One score matmul per (pair, sk-tile): out[sk=128, 2*512] = krotz.T @
    qpack[:, pair-heads, q-tile] (K=128 with zero bottom half).  Score PSUM
    pairs S0/S1 double-buffer across iterations so the EXP stream on ACT never
    waits on PE.  probsT = exp(scale*s) (one ACT instr per pair-iter, fp32
    psum -> fp16); diagonal blocks masked by one 0/1-mask multiply (DVE).
    PV: one matmul out[65, 1024] = [V|1].T @ probs accumulates over sk-tiles
    (row 64 = softmax denominator).  Normalize: DVE copy of denom row,
    reciprocal_approx_fast, gpsimd partition-broadcast, DVE multiply -> a0/a1.
  phase 3  output projection, interleaved into phase 2: yo[128, 2, 512] =
    [a0|a1].T-slices @ wo in the transiently-free PV psum banks (throttled so
    a 2-tile reserve keeps the PE warm through the final normalize), cast to
    fp16 (DVE; +ACT in the drain loop), DMA'd to y; host sums the 8 per-core
    partial y in fp32.
"""

import contextlib
from collections import deque
import numpy as np
import jax.numpy as jnp

import concourse.bass as bass
import concourse.tile as tile
from concourse import bacc, mybir
from concourse.masks import make_identity
import concourse.dve_ops as _dops
from concourse.dve_spec import Spec, Src0, C0, C1


def _ref_exp_schrau(in0, in1, c0, c1, c2):
    return in0 * c0 + c1


def _register_exp_approx():
    """Schraudolph exp on the DVE: bitcast16(int16(A*x + B)) ~ exp(x*scale).
    One 2-stage custom op, 1 elem/cycle/lane -- lets the Vector engine absorb
    a slice of the softmax-exp stream that otherwise saturates ScalarE.
    Registered once per process via the documented dve_ops extension point."""
    for op in _dops.OPS:
        if op.name == "EXP_APPROX_SCHRAU":
            return op
    op = _dops.DveOp("EXP_APPROX_SCHRAU",
                     Spec(body=Src0 * C0 + C1, reference=_ref_exp_schrau),
                     subdim=False,
                     uops_sha={"v3": "2230da7084b02538", "v4": None})
    _dops.OPS.append(op)
    _dops.CUSTOM_DVE_SPECS[op.name] = op.spec
    _dops._SUB_OPCODE_FOR_NAME[op.name] = (
        _dops._CUSTOM_DVE_ROW_BASE + len(_dops.OPS) - 1)
    return op


EXP_APPROX = _register_exp_approx()
# fp16 Schraudolph constants; score scale folded into the multiplier
EXP_A16 = 1024.0 / np.log(2.0)
EXP_B16 = 15.0 * 1024.0 - 44.7

B, S, D = 2, 2048, 2048
H, KV, HD = 32, 8, 64
T = B * S
NCORES = 8
HPC = H // NCORES          # 4 query heads per core
SCALE = 1.0 / np.sqrt(HD)
THETA = 10000.0
NQT = T // 512             # 8 token tiles of 512
REPLICATED = {"xT", "cos4", "sin4", "mask"}  # same bytes on every core
NDT = D // 128             # 16 contraction tiles
F32 = mybir.dt.float32
F16 = mybir.dt.float16


def _build_program():
    nc = bacc.Bacc("TRN2", target_bir_lowering=False, debug=False)

    xT = nc.dram_tensor("xT", [D, T], F16, kind="ExternalInput")
    wq = nc.dram_tensor("wq", [D, 2 * HPC * 32], F16, kind="ExternalInput")
    wkv = nc.dram_tensor("wkv", [D, 128], F16, kind="ExternalInput")
    wo = nc.dram_tensor("wo", [HPC * HD, D], F16, kind="ExternalInput")
    cos4 = nc.dram_tensor("cos4", [128, S], F32, kind="ExternalInput")
    sin4 = nc.dram_tensor("sin4", [128, S], F32, kind="ExternalInput")
    maskd = nc.dram_tensor("mask", [128, 896], F16, kind="ExternalInput")
    y = nc.dram_tensor("y", [T, D], F16, kind="ExternalOutput")

    with tile.TileContext(nc) as tc:
        _body(tc, nc, xT, wq, wkv, wo, cos4, sin4, maskd, y)
    nc.compile()
    return nc


def _body(tc, nc, xT, wq, wkv, wo, cos4, sin4, maskd, y):
    TT = mybir.AluOpType
    # PSUM = 8 banks of [128 x 512 f32], as four 2-bank pairs:
    #   S0/S1: phase-1 qa/qb pairs; phase-2 score tiles (double-buffered).
    #   PV0/PV1: phase-1 kv + V-transpose scratch; phase-2 PV accumulators
    #            (one per head-pair); phase-3 yo tiles (in the windows where
    #            the other pair's accumulator is idle).
    ctx = contextlib.ExitStack()
    with ctx:
        const = ctx.enter_context(tc.tile_pool(name="const", bufs=1))
        persist = ctx.enter_context(tc.tile_pool(name="persist", bufs=1))
        xs = ctx.enter_context(tc.tile_pool(name="xs", bufs=6))
        rtmp = ctx.enter_context(tc.tile_pool(name="rtmp", bufs=2))
        probs = ctx.enter_context(tc.tile_pool(name="probs", bufs=3))
        norm = ctx.enter_context(tc.tile_pool(name="norm", bufs=2))
        yst = ctx.enter_context(tc.tile_pool(name="yst", bufs=3))
        psum = ctx.enter_context(tc.tile_pool(name="psum", bufs=1, space="PSUM"))

        # ---- constants ----
        # Constants go down the ACT HWDGE ring so the phase-1 x-tile stream
        # owns the SP ring exclusively.  wo is phase-3-only: last.
        wq_sbs, wkv_sbs = [], []
        wqr = wq[:, :].rearrange("(t p) c -> p t c", p=128)
        wkvr = wkv[:, :].rearrange("(t p) c -> p t c", p=128)
        for hnum in range(4):
            sl = slice(hnum * 4, (hnum + 1) * 4)
            wq_h = const.tile([128, 4, 256], F16, name=f"wq_sb{hnum}")
            nc.scalar.dma_start(out=wq_h, in_=wqr[:, sl, :])
            wq_sbs.append(wq_h)
            wkv_h = const.tile([128, 4, 128], F16, name=f"wkv_sb{hnum}")
            nc.scalar.dma_start(out=wkv_h, in_=wkvr[:, sl, :])
            wkv_sbs.append(wkv_h)
        cos_sb = const.tile([128, S], F32, name="cos_sb")
        nc.scalar.dma_start(out=cos_sb, in_=cos4[:, :])
        sin_sb = const.tile([128, S], F32, name="sin_sb")
        nc.scalar.dma_start(out=sin_sb, in_=sin4[:, :])
        # mask duplicated per head-pair half so one DVE op masks both heads
        mask2 = const.tile([128, 2, 896], F16, name="mask2")
        nc.scalar.dma_start(out=mask2[:, 0, :], in_=maskd[:, :])
        nc.scalar.dma_start(out=mask2[:, 1, :], in_=maskd[:, :])
        wo_sb = const.tile([128, 2, D], F16, name="wo_sb")
        nc.scalar.dma_start(out=wo_sb, in_=wo[:, :].rearrange("(t p) c -> p t c", p=128))
        ident = const.tile([64, 64], F16, name="ident")
        make_identity(nc, ident)

        # ---- persistent activations ----
        # qpack[(h%2)*64:(h%2)*64+64, h//2, tok] = rope'd q of head h.  The
        # score matmuls contract K=128 with zero-padded K weights (kz = [k;0],
        # zk = [0;k]) so the off-head rows annihilate; K=64 matmuls would
        # mode-switch/drain the PE (and risk the HAM clock gate).
        qpack = persist.tile([128, 2, T], F16, name="qpack")
        krotz = persist.tile([128, T], F16, name="krotz")    # rows 0:64=k', 64:128=0
        zkrot = persist.tile([128, T], F16, name="zkrot")    # rows 0:64=0, 64:128=k'
        nc.gpsimd.memset(krotz[64:128, :], 0.0)
        nc.gpsimd.memset(zkrot[0:64, :], 0.0)
        vnat = persist.tile([128, T // 128, 65], F16, name="vnat")  # [tok%128, toktile, hd+1]
        a0 = persist.tile([128, T], F16, name="a0")          # attn outT, heads 0,1
        a1 = persist.tile([128, T], F16, name="a1")          # attn outT, heads 2,3
        ones_c = const.tile([128, T // 128, 1], F16, name="ones_c")
        nc.vector.memset(ones_c, 1.0)
        nc.vector.tensor_copy(out=vnat[:, :, 64:65], in_=ones_c)

        # ================= phase 1: projections + rope =================
        SNM = ["S0", "S1"]
        PNM = ["PV0", "PV1"]
        for qt in range(NQT):
            pos0 = (qt % 4) * 512
            tok0 = qt * 512
            qab = psum.tile([128, 2, 512], F32, name=SNM[qt % 2])
            kvp = psum.tile([128, 2, 512], F32, name=PNM[qt % 2])
            kv_ps = kvp[:, 0, :]
            qa_ps = qab[:, 0, :]
            qb_ps = qab[:, 1, :]
            xt4s = []
            for d4 in range(4):
                xt4 = xs.tile([128, 4, 512], F16, name="xt4")
                nc.sync.dma_start(
                    out=xt4,
                    in_=xT[d4 * 512:(d4 + 1) * 512, tok0:tok0 + 512]
                    .rearrange("(t p) c -> p t c", p=128))
                xt4s.append(xt4)
            for d in range(NDT):
                xt = xt4s[d // 4][:, d % 4, :]
                st, sp = d == 0, d == NDT - 1
                wq_d = wq_sbs[d // 4][:, d % 4, :]
                nc.tensor.matmul(out=qa_ps, lhsT=(wq_d[:, 0:128]), rhs=(xt),
                                 start=st, stop=sp)
                nc.tensor.matmul(out=qb_ps, lhsT=(wq_d[:, 128:256]), rhs=(xt),
                                 start=st, stop=sp)
                nc.tensor.matmul(out=kv_ps, lhsT=(wkv_sbs[d // 4][:, d % 4, :]), rhs=(xt),
                                 start=st, stop=sp)
            cs = cos_sb[:, pos0:pos0 + 512]
            sn = sin_sb[:, pos0:pos0 + 512]
            # V copy first on DVE: the PE transposes wait only on it.
            vt = rtmp.tile([64, 512], F16, name="vt")
            nc.vector.tensor_copy(out=vt, in_=kvp[64:128, 0, :])
            for k4 in range(4):
                tpv = kvp[:, 1, k4 * 32:(k4 + 1) * 32].bitcast(F16)
                nc.tensor.transpose(tpv, vt[:, k4 * 128:(k4 + 1) * 128], ident)
            # K rope (single kv head): rows 0:32 ka, 32:64 kb of kv.
            k_x = rtmp.tile([32, 512], F32, name="k_x")
            k_x2 = rtmp.tile([32, 512], F32, name="k_x2")
            k_y = rtmp.tile([32, 512], F32, name="k_y")
            k_y2 = rtmp.tile([32, 512], F32, name="k_y2")
            # Q rope on [128, 512] (row 32h+r = head h dim r); both reads of
            # each psum issued back-to-back so the bank frees early.
            t_x = rtmp.tile([128, 512], F32, name="t_x")
            t_x2 = rtmp.tile([128, 512], F32, name="t_x2")
            nc.vector.tensor_tensor(out=t_x, in0=qa_ps, in1=cs, op=TT.mult)
            nc.vector.tensor_tensor(out=t_x2, in0=qa_ps, in1=sn, op=TT.mult)
            t_y = rtmp.tile([128, 512], F32, name="t_y")
            t_y2 = rtmp.tile([128, 512], F32, name="t_y2")
            nc.vector.tensor_tensor(out=t_y, in0=qb_ps, in1=sn, op=TT.mult)
            nc.vector.tensor_tensor(out=t_y2, in0=qb_ps, in1=cs, op=TT.mult)
            qra = rtmp.tile([128, 512], F16, name="qra")
            qrb = rtmp.tile([128, 512], F16, name="qrb")
            nc.vector.tensor_tensor(out=qra, in0=t_x, in1=t_y, op=TT.subtract)
            nc.vector.tensor_tensor(out=qrb, in0=t_x2, in1=t_y2, op=TT.add)
            nc.vector.tensor_tensor(out=k_x, in0=kvp[0:32, 0, :], in1=cs[0:32], op=TT.mult)
            nc.vector.tensor_tensor(out=k_x2, in0=kvp[0:32, 0, :], in1=sn[0:32], op=TT.mult)
            nc.vector.tensor_tensor(out=k_y, in0=kvp[32:64, 0, :], in1=sn[0:32], op=TT.mult)
            nc.vector.tensor_tensor(out=k_y2, in0=kvp[32:64, 0, :], in1=cs[0:32], op=TT.mult)
            # remap q into [pair-half row, pair, tok] layout for row-tiled scores
            for h in range(HPC):
                rb = (h % 2) * 64
                nc.scalar.dma_start(out=qpack[rb:rb + 32, h // 2, tok0:tok0 + 512],
                                    in_=qra[32 * h:32 * h + 32, :])
                nc.scalar.dma_start(out=qpack[rb + 32:rb + 64, h // 2, tok0:tok0 + 512],
                                    in_=qrb[32 * h:32 * h + 32, :])
            nc.vector.tensor_tensor(out=krotz[0:32, tok0:tok0 + 512], in0=k_x,
                                    in1=k_y, op=TT.subtract)
            nc.vector.tensor_tensor(out=krotz[32:64, tok0:tok0 + 512], in0=k_x2,
                                    in1=k_y2, op=TT.add)
            nc.gpsimd.tensor_copy(out=zkrot[64:128, tok0:tok0 + 512],
                                  in_=krotz[0:64, tok0:tok0 + 512])
            for k4 in range(4):
                tpv = kvp[:, 1, k4 * 32:(k4 + 1) * 32].bitcast(F16)
                nc.vector.tensor_copy(out=vnat[:, qt * 4 + k4, 0:64], in_=tpv)

        # ================= phase 2: attention (+ phase 3 interleaved) ======
        # yo tiles are emitted into the PV pair that is idle (the other
        # head-pair's accumulator), budgeted so the pair is free again before
        # the next (b, jq) needs it.
        pending = deque()   # (tt, nh) output tiles owed
        proj_ct = [0]
        drain = [False]  # in the drain loop ACT is idle: split casts across engines

        def emit_proj(pname):
            tt, nh = pending.popleft()
            yo = psum.tile([128, 2, 512], F32, name=pname)
            for half in range(2):
                n = nh * 2 + half
                nc.tensor.matmul(out=yo[:, half, :],
                                 lhsT=(a0[:, tt * 128:(tt + 1) * 128]),
                                 rhs=(wo_sb[:, 0, n * 512:(n + 1) * 512]),
                                 start=True, stop=False)
                nc.tensor.matmul(out=yo[:, half, :],
                                 lhsT=(a1[:, tt * 128:(tt + 1) * 128]),
                                 rhs=(wo_sb[:, 1, n * 512:(n + 1) * 512]),
                                 start=False, stop=True)
            stage = yst.tile([128, 2, 512], F16, name="stage")
            # in-loop casts stay on the DVE: a copy on ScalarE would queue
            # ahead of the next EXP in its FIFO and stall the score pipeline
            if drain[0] and proj_ct[0] % 2 == 1:
                nc.scalar.copy(out=stage, in_=yo)
            else:
                nc.vector.tensor_copy(out=stage, in_=yo)
            proj_ct[0] += 1
            nc.sync.dma_start(out=y[tt * 128:(tt + 1) * 128,
                                    nh * 1024:(nh + 1) * 1024], in_=stage)

        si = 0
        unit = 0
        for b in range(B):
            for jq in range(4):
                tq = b * S + jq * 512
                ni = 4 * jq + 4
                for pair in range(2):
                    pvp = psum.tile([65, 2, 512], F32, name=PNM[pair])
                    pend = None  # probs tile not yet fed to PV
                    for i in range(ni):
                        tk = b * S + i * 128
                        sc = psum.tile([128, 2, 512], F32, name=SNM[si % 2])
                        si += 1
                        for h2, kt in ((0, krotz), (1, zkrot)):
                            nc.tensor.matmul(
                                out=sc[:, h2, :], lhsT=(kt[:, tk:tk + 128]),
                                rhs=(qpack[:, pair, tq:tq + 512]),
                                start=True, stop=True)
                        if pend is not None:
                            ip, ptp = pend
                            for h2 in range(2):
                                nc.tensor.matmul(out=pvp[:, h2, :],
                                                 lhsT=(vnat[:, b * 16 + ip, :]),
                                                 rhs=(ptp[:, h2, :]),
                                                 start=ip == 0, stop=False)
                        if len(pending) > 2 and i % 2 == 1:
                            emit_proj(PNM[1 - pair])
                        pt = probs.tile([128, 2, 512], F16, name="pt")
                        nc.scalar.activation(out=pt, in_=sc,
                                             func=mybir.ActivationFunctionType.Exp,
                                             scale=float(SCALE))
                        unit += 1
                        if i >= 4 * jq:  # diagonal block: causal mask
                            roff = 128 * i - 512 * jq
                            nc.vector.tensor_tensor(
                                out=pt, in0=pt,
                                in1=mask2[:, :, 384 - roff:896 - roff], op=TT.mult)
                        pend = (i, pt)
                    ip, ptp = pend
                    for h2 in range(2):
                        nc.tensor.matmul(out=pvp[:, h2, :],
                                         lhsT=(vnat[:, b * 16 + ip, :]),
                                         rhs=(ptp[:, h2, :]),
                                         start=ip == 0, stop=True)
                    # normalize: row 64 of pvp is the softmax denominator.
                    # (the custom-DVE reciprocal reads garbage from PSUM on
                    # HW -- bounce the denominator row through SBUF first.)
                    sums = norm.tile([1, 2, 512], F32, name="sums")
                    nc.vector.tensor_copy(out=sums, in_=pvp[64:65, :, :])
                    rec = norm.tile([1, 2, 512], F32, name="rec")
                    nc.vector.reciprocal_approx_fast(out=rec, in_=sums)
                    dst = a0 if pair == 0 else a1
                    for h2 in range(2):
                        rbc = norm.tile([64, 512], F32, name="rbc")
                        nc.gpsimd.partition_broadcast(rbc, rec[0:1, h2, :])
                        nc.vector.tensor_tensor(out=dst[h2 * 64:h2 * 64 + 64, tq:tq + 512],
                                                in0=pvp[0:64, h2, :], in1=rbc,
                                                op=TT.mult)
                # output tiles of this (b, jq) are complete after both pairs
                tt0 = b * 16 + jq * 4
                pending.extend((tt0 + t, nh) for t in range(4) for nh in range(2))

        # remaining projections rotate through all four freed psum pairs
        drain[0] = True
        k = 0
        while pending:
            emit_proj((SNM + PNM)[k % 4])
            k += 1


_CACHE = {}


def _get_program():
    if "nc" not in _CACHE:
        _CACHE["nc"] = _build_program()
    return _CACHE["nc"]


def _get_runner():
    """Cached jitted shard_map executable over 8 cores (avoids per-call
    retrace that run_bass_kernel_spmd pays)."""
    if "runner" in _CACHE:
        return _CACHE["runner"]
    import jax
    from jax.sharding import Mesh, PartitionSpec
    from jax.experimental.shard_map import shard_map
    from concourse import bass2jax
    from concourse.bass2jax import _bass_exec_p

    bass2jax.install_neuronx_cc_hook()
    nc = _get_program()
    partition_name = nc.partition_id_tensor.name if nc.partition_id_tensor else None
    in_names, out_names, out_avals = [], [], []
    for alloc in nc.m.functions[0].allocations:
        if not isinstance(alloc, mybir.MemoryLocationSet):
            continue
        name = alloc.memorylocations[0].name
        if alloc.kind == "ExternalInput":
            if name != partition_name:
                in_names.append(name)
        elif alloc.kind == "ExternalOutput":
            out_names.append(name)
            out_avals.append(jax.core.ShapedArray(
                tuple(alloc.tensor_shape), mybir.dt.np(alloc.dtype)))
    n_params = len(in_names)
    n_outs = len(out_avals)
    all_in = list(in_names) + list(out_names)
    if partition_name is not None:
        all_in.append(partition_name)

    def _body_fn(*args):
        operands = list(args)
        if partition_name is not None:
            operands.append(bass2jax.partition_id_tensor())
        return tuple(_bass_exec_p.bind(
            *operands,
            out_avals=tuple(out_avals),
            in_names=tuple(all_in),
            out_names=tuple(out_names),
            lowering_input_output_aliases=(),
            sim_require_finite=True,
            sim_require_nnan=True,
            nc=nc,
        ))

    devices = jax.devices()[:NCORES]
    mesh = Mesh(np.asarray(devices), ("core",))
    # xT / rope tables / mask are identical on every core: feed them
    # replicated (P()) so the host uploads one copy + on-device all-gather,
    # instead of 8 copies through the tunnel.
    in_specs = tuple(
        PartitionSpec() if n in REPLICATED else PartitionSpec("core")
        for n in in_names) + (PartitionSpec("core"),) * n_outs
    sharded = jax.jit(
        shard_map(_body_fn, mesh=mesh,
                  in_specs=in_specs,
                  out_specs=(PartitionSpec("core"),) * n_outs,
                  check_rep=False),
        donate_argnums=tuple(range(n_params, n_params + n_outs)),
        keep_unused=True)

    from jax.sharding import NamedSharding
    rep = NamedSharding(mesh, PartitionSpec())
    shd = NamedSharding(mesh, PartitionSpec("core"))
    gather = jax.jit(lambda a: a, out_shardings=rep)   # upload-shard -> all-gather
    zeros = jax.jit(lambda: jnp.zeros((NCORES * T, D), jnp.float16),
                    out_shardings=shd)
    reduce_y = jax.jit(lambda yc: yc.reshape(NCORES, T, D)
                       .sum(0, dtype=jnp.float32), out_shardings=rep)
    _CACHE["runner"] = (sharded, in_names, out_names, out_avals,
                        mesh, rep, shd, gather, zeros, reduce_y)
    return _CACHE["runner"]


def _host_inputs(x, wq, wk, wv, wo):
    x = np.asarray(x, np.float32)
    wq = np.asarray(wq, np.float16)
    wk = np.asarray(wk, np.float16)
    wv = np.asarray(wv, np.float16)
    wo = np.asarray(wo, np.float16)

    xT = np.ascontiguousarray(x.reshape(T, D).T.astype(np.float16))

    inv = 1.0 / (THETA ** (np.arange(0, HD, 2, dtype=np.float64) / HD))
    fr = np.outer(inv, np.arange(S, dtype=np.float64))   # [32, S]
    cosT = np.cos(fr).astype(np.float32)
    sinT = np.sin(fr).astype(np.float32)
    cos4 = np.ascontiguousarray(np.tile(cosT, (4, 1)))
    sin4 = np.ascontiguousarray(np.tile(sinT, (4, 1)))

    u = np.arange(896)[None, :]
    p = np.arange(128)[:, None]
    mask = (u >= p + 384).astype(np.float16)

    in_maps = []
    for c in range(NCORES):
        cols_a, cols_b = [], []
        for h in range(HPC):
            base = (HPC * c + h) * HD
            cols_a.append(wq[:, base:base + 32])
            cols_b.append(wq[:, base + 32:base + 64])
        wq_c = np.ascontiguousarray(np.concatenate(cols_a + cols_b, axis=1))
        kb = c * HD
        wkv_c = np.ascontiguousarray(np.concatenate(
            [wk[:, kb:kb + 32], wk[:, kb + 32:kb + 64], wv[:, kb:kb + HD]], axis=1))
        wo_c = np.ascontiguousarray(wo[c * HPC * HD:(c + 1) * HPC * HD, :])
        in_maps.append({"xT": xT, "wq": wq_c, "wkv": wkv_c, "wo": wo_c,
                        "cos4": cos4, "sin4": sin4, "mask": mask})
    return in_maps


def _stage_inputs(in_maps):
    """Upload inputs: replicated tensors go up as 1/8 shards and are
    all-gathered on device; per-core tensors upload as the usual concat."""
    import jax
    (sharded, in_names, out_names, out_avals,
     mesh, rep, shd, gather, zeros, reduce_y) = _get_runner()
    staged = []
    for n in in_names:
        if n in REPLICATED:
            a = in_maps[0][n]
            if a.shape[0] % NCORES == 0:
                staged.append(gather(jax.device_put(a, shd)))
            else:
                staged.append(jax.device_put(a, rep))
        else:
            cat = np.concatenate([m[n] for m in in_maps], axis=0)
            staged.append(jax.device_put(cat, shd))
    return staged


def kernel(x, wq, wk, wv, wo):
    import jax
    (sharded, in_names, out_names, out_avals,
     mesh, rep, shd, gather, zeros, reduce_y) = _get_runner()
    in_maps = _host_inputs(x, wq, wk, wv, wo)
    staged = _stage_inputs(in_maps)
    out_arrs = sharded(*staged, zeros())
    ysum = reduce_y(out_arrs[out_names.index("y")])
    return np.asarray(ysum).reshape(B, S, D)


# revision 15
# speedup vs baseline: 1.0366x; 1.0133x over previous
"""Grouped-Query Attention (B=2, S=2048, D=2048, H=32, KV=8, HD=64) on 8 TRN2
NeuronCores, tensor-parallel over KV-head groups (1 KV head + 4 Q heads per
core), with host-side shard/gather.

All matmul operands are fp16 (PE streams 1 cyc/row; PSUM accumulation fp32),
elementwise RoPE/softmax math fp32 where it reads PSUM.

Per-core dataflow (activations kept feature-on-partitions so every matmul
contracts over the partition dim with no on-device transposition of x):

  phase 1  QKV projection + RoPE
    xT d-tiles stream in batches of 4 (one DMA each) -> psum: qa/qb (S pair),
    kv (PV pair); RoPE on DVE from PSUM; Q written to qpack[0:64, h, tok]
    (rows 64:128 stay zero -- they face zero K-weights in the score matmul);
    K written to krotz rows 0:64 (rows 64:128 zero); V transposed back to
    natural [tok, hd] via PE transpose, + a ones column (softmax denominator).
  phase 2  attention per (batch, q-tile 512, head-pair), causal-block-skipped
    One score matmul per (pair, sk-tile): out[sk=128, 2*512] = krotz.T @
    qpack[:, pair-heads, q-tile] (K=128 with zero bottom half).  Score PSUM
    pairs S0/S1 double-buffer across iterations so the EXP stream on ACT never
    waits on PE.  probsT = exp(scale*s) (one ACT instr per pair-iter, fp32
    psum -> fp16); diagonal blocks masked by one 0/1-mask multiply (DVE).
    PV: one matmul out[65, 1024] = [V|1].T @ probs accumulates over sk-tiles
    (row 64 = softmax denominator).  Normalize: DVE copy of denom row,
    reciprocal_approx_fast, gpsimd partition-broadcast, DVE multiply -> a0/a1.
  phase 3  output projection, interleaved into phase 2: yo[128, 2, 512] =
    [a0|a1].T-slices @ wo in the transiently-free PV psum banks (throttled so
    a 2-tile reserve keeps the PE warm through the final normalize), cast to
    fp16 (DVE; +ACT in the drain loop), DMA'd to y; host sums the 8 per-core
    partial y in fp32.
"""

import contextlib
from collections import deque
import numpy as np
import jax.numpy as jnp

import concourse.bass as bass
import concourse.tile as tile
from concourse import bacc, mybir
from concourse.masks import make_identity
import concourse.dve_ops as _dops
from concourse.dve_spec import Spec, Src0, C0, C1


def _ref_exp_schrau(in0, in1, c0, c1, c2):
    return in0 * c0 + c1


def _register_exp_approx():
    """Schraudolph exp on the DVE: bitcast16(int16(A*x + B)) ~ exp(x*scale).
    One 2-stage custom op, 1 elem/cycle/lane -- lets the Vector engine absorb
    a slice of the softmax-exp stream that otherwise saturates ScalarE.
    Registered once per process via the documented dve_ops extension point."""
    for op in _dops.OPS:
        if op.name == "EXP_APPROX_SCHRAU":
            return op
    op = _dops.DveOp("EXP_APPROX_SCHRAU",
                     Spec(body=Src0 * C0 + C1, reference=_ref_exp_schrau),
                     subdim=False,
                     uops_sha={"v3": "2230da7084b02538", "v4": None})
    _dops.OPS.append(op)
    _dops.CUSTOM_DVE_SPECS[op.name] = op.spec
    _dops._SUB_OPCODE_FOR_NAME[op.name] = (
        _dops._CUSTOM_DVE_ROW_BASE + len(_dops.OPS) - 1)
    return op


EXP_APPROX = _register_exp_approx()
# fp16 Schraudolph constants; score scale folded into the multiplier
EXP_A16 = 1024.0 / np.log(2.0)
EXP_B16 = 15.0 * 1024.0 - 44.7

B, S, D = 2, 2048, 2048
H, KV, HD = 32, 8, 64
T = B * S
NCORES = 8
HPC = H // NCORES          # 4 query heads per core
SCALE = 1.0 / np.sqrt(HD)
THETA = 10000.0
NQT = T // 512             # 8 token tiles of 512
REPLICATED = {"xT", "cos4", "sin4", "mask"}  # same bytes on every core
NDT = D // 128             # 16 contraction tiles
F32 = mybir.dt.float32
F16 = mybir.dt.float16


def _build_program():
    nc = bacc.Bacc("TRN2", target_bir_lowering=False, debug=False)

    xT = nc.dram_tensor("xT", [D, T], F16, kind="ExternalInput")
    wq = nc.dram_tensor("wq", [D, 2 * HPC * 32], F16, kind="ExternalInput")
    wkv = nc.dram_tensor("wkv", [D, 128], F16, kind="ExternalInput")
    wo = nc.dram_tensor("wo", [HPC * HD, D], F16, kind="ExternalInput")
    cos4 = nc.dram_tensor("cos4", [128, S], F32, kind="ExternalInput")
    sin4 = nc.dram_tensor("sin4", [128, S], F32, kind="ExternalInput")
    maskd = nc.dram_tensor("mask", [128, 896], F16, kind="ExternalInput")
    y = nc.dram_tensor("y", [T, D], F16, kind="ExternalOutput")

    with tile.TileContext(nc) as tc:
        _body(tc, nc, xT, wq, wkv, wo, cos4, sin4, maskd, y)
    nc.compile()
    return nc


def _body(tc, nc, xT, wq, wkv, wo, cos4, sin4, maskd, y):
    TT = mybir.AluOpType
    # PSUM = 8 banks of [128 x 512 f32], as four 2-bank pairs:
    #   S0/S1: phase-1 qa/qb pairs; phase-2 score tiles (double-buffered).
    #   PV0/PV1: phase-1 kv + V-transpose scratch; phase-2 PV accumulators
    #            (one per head-pair); phase-3 yo tiles (in the windows where
    #            the other pair's accumulator is idle).
    ctx = contextlib.ExitStack()
    with ctx:
        const = ctx.enter_context(tc.tile_pool(name="const", bufs=1))
        persist = ctx.enter_context(tc.tile_pool(name="persist", bufs=1))
        xs = ctx.enter_context(tc.tile_pool(name="xs", bufs=6))
        rtmp = ctx.enter_context(tc.tile_pool(name="rtmp", bufs=2))
        probs = ctx.enter_context(tc.tile_pool(name="probs", bufs=4))
        norm = ctx.enter_context(tc.tile_pool(name="norm", bufs=2))
        yst = ctx.enter_context(tc.tile_pool(name="yst", bufs=4))
        psum = ctx.enter_context(tc.tile_pool(name="psum", bufs=1, space="PSUM"))

        # ---- constants ----
        # Constants go down the ACT HWDGE ring so the phase-1 x-tile stream
        # owns the SP ring exclusively.  wo is phase-3-only: last.
        wq_sbs, wkv_sbs = [], []
        wqr = wq[:, :].rearrange("(t p) c -> p t c", p=128)
        wkvr = wkv[:, :].rearrange("(t p) c -> p t c", p=128)
        for hnum in range(4):
            sl = slice(hnum * 4, (hnum + 1) * 4)
            wq_h = const.tile([128, 4, 256], F16, name=f"wq_sb{hnum}")
            nc.scalar.dma_start(out=wq_h, in_=wqr[:, sl, :])
            wq_sbs.append(wq_h)
            wkv_h = const.tile([128, 4, 128], F16, name=f"wkv_sb{hnum}")
            nc.scalar.dma_start(out=wkv_h, in_=wkvr[:, sl, :])
            wkv_sbs.append(wkv_h)
        cos_sb = const.tile([128, S], F32, name="cos_sb")
        nc.scalar.dma_start(out=cos_sb, in_=cos4[:, :])
        sin_sb = const.tile([128, S], F32, name="sin_sb")
        nc.scalar.dma_start(out=sin_sb, in_=sin4[:, :])
        # mask duplicated per head-pair half so one DVE op masks both heads
        mask2 = const.tile([128, 2, 896], F16, name="mask2")
        nc.scalar.dma_start(out=mask2[:, 0, :], in_=maskd[:, :])
        nc.scalar.dma_start(out=mask2[:, 1, :], in_=maskd[:, :])
        wo_sb = const.tile([128, 2, D], F16, name="wo_sb")
        nc.scalar.dma_start(out=wo_sb, in_=wo[:, :].rearrange("(t p) c -> p t c", p=128))
        ident = const.tile([64, 64], F16, name="ident")
        make_identity(nc, ident)

        # ---- persistent activations ----
        # qpack[(h%2)*64:(h%2)*64+64, h//2, tok] = rope'd q of head h.  The
        # score matmuls contract K=128 with zero-padded K weights (kz = [k;0],
        # zk = [0;k]) so the off-head rows annihilate; K=64 matmuls would
        # mode-switch/drain the PE (and risk the HAM clock gate).
        qpack = persist.tile([128, 2, T], F16, name="qpack")
        krotz = persist.tile([128, T], F16, name="krotz")    # rows 0:64=k', 64:128=0
        zkrot = persist.tile([128, T], F16, name="zkrot")    # rows 0:64=0, 64:128=k'
        nc.gpsimd.memset(krotz[64:128, :], 0.0)
        nc.gpsimd.memset(zkrot[0:64, :], 0.0)
        vnat = persist.tile([128, T // 128, 65], F16, name="vnat")  # [tok%128, toktile, hd+1]
        a0 = persist.tile([128, T], F16, name="a0")          # attn outT, heads 0,1
        a1 = persist.tile([128, T], F16, name="a1")          # attn outT, heads 2,3
        ones_c = const.tile([128, T // 128, 1], F16, name="ones_c")
        nc.vector.memset(ones_c, 1.0)
        nc.vector.tensor_copy(out=vnat[:, :, 64:65], in_=ones_c)

        # ================= phase 1: projections + rope =================
        SNM = ["S0", "S1"]
        PNM = ["PV0", "PV1"]
        for qt in range(NQT):
            pos0 = (qt % 4) * 512
            tok0 = qt * 512
            qab = psum.tile([128, 2, 512], F32, name=SNM[qt % 2])
            kvp = psum.tile([128, 2, 512], F32, name=PNM[qt % 2])
            kv_ps = kvp[:, 0, :]
            qa_ps = qab[:, 0, :]
            qb_ps = qab[:, 1, :]
            xt4s = []
            for d4 in range(4):
                xt4 = xs.tile([128, 4, 512], F16, name="xt4")
                nc.sync.dma_start(
                    out=xt4,
                    in_=xT[d4 * 512:(d4 + 1) * 512, tok0:tok0 + 512]
                    .rearrange("(t p) c -> p t c", p=128))
                xt4s.append(xt4)
            for d in range(NDT):
                xt = xt4s[d // 4][:, d % 4, :]
                st, sp = d == 0, d == NDT - 1
                wq_d = wq_sbs[d // 4][:, d % 4, :]
                nc.tensor.matmul(out=qa_ps, lhsT=(wq_d[:, 0:128]), rhs=(xt),
                                 start=st, stop=sp)
                nc.tensor.matmul(out=qb_ps, lhsT=(wq_d[:, 128:256]), rhs=(xt),
                                 start=st, stop=sp)
                nc.tensor.matmul(out=kv_ps, lhsT=(wkv_sbs[d // 4][:, d % 4, :]), rhs=(xt),
                                 start=st, stop=sp)
            cs = cos_sb[:, pos0:pos0 + 512]
            sn = sin_sb[:, pos0:pos0 + 512]
            # V copy first on DVE: the PE transposes wait only on it.
            vt = rtmp.tile([64, 512], F16, name="vt")
            nc.vector.tensor_copy(out=vt, in_=kvp[64:128, 0, :])
            for k4 in range(4):
                tpv = kvp[:, 1, k4 * 32:(k4 + 1) * 32].bitcast(F16)
                nc.tensor.transpose(tpv, vt[:, k4 * 128:(k4 + 1) * 128], ident)
            # K rope (single kv head): rows 0:32 ka, 32:64 kb of kv.
            k_x = rtmp.tile([32, 512], F32, name="k_x")
            k_x2 = rtmp.tile([32, 512], F32, name="k_x2")
            k_y = rtmp.tile([32, 512], F32, name="k_y")
            k_y2 = rtmp.tile([32, 512], F32, name="k_y2")
            # Q rope on [128, 512] (row 32h+r = head h dim r); both reads of
            # each psum issued back-to-back so the bank frees early.
            t_x = rtmp.tile([128, 512], F32, name="t_x")
            t_x2 = rtmp.tile([128, 512], F32, name="t_x2")
            nc.vector.tensor_tensor(out=t_x, in0=qa_ps, in1=cs, op=TT.mult)
            nc.vector.tensor_tensor(out=t_x2, in0=qa_ps, in1=sn, op=TT.mult)
            t_y = rtmp.tile([128, 512], F32, name="t_y")
            t_y2 = rtmp.tile([128, 512], F32, name="t_y2")
            nc.vector.tensor_tensor(out=t_y, in0=qb_ps, in1=sn, op=TT.mult)
            nc.vector.tensor_tensor(out=t_y2, in0=qb_ps, in1=cs, op=TT.mult)
            qra = rtmp.tile([128, 512], F16, name="qra")
            qrb = rtmp.tile([128, 512], F16, name="qrb")
            nc.vector.tensor_tensor(out=qra, in0=t_x, in1=t_y, op=TT.subtract)
            nc.vector.tensor_tensor(out=qrb, in0=t_x2, in1=t_y2, op=TT.add)
            nc.vector.tensor_tensor(out=k_x, in0=kvp[0:32, 0, :], in1=cs[0:32], op=TT.mult)
            nc.vector.tensor_tensor(out=k_x2, in0=kvp[0:32, 0, :], in1=sn[0:32], op=TT.mult)
            nc.vector.tensor_tensor(out=k_y, in0=kvp[32:64, 0, :], in1=sn[0:32], op=TT.mult)
            nc.vector.tensor_tensor(out=k_y2, in0=kvp[32:64, 0, :], in1=cs[0:32], op=TT.mult)
            # remap q into [pair-half row, pair, tok] layout for row-tiled scores
            for h in range(HPC):
                rb = (h % 2) * 64
                nc.scalar.dma_start(out=qpack[rb:rb + 32, h // 2, tok0:tok0 + 512],
                                    in_=qra[32 * h:32 * h + 32, :])
                nc.scalar.dma_start(out=qpack[rb + 32:rb + 64, h // 2, tok0:tok0 + 512],
                                    in_=qrb[32 * h:32 * h + 32, :])
            nc.vector.tensor_tensor(out=krotz[0:32, tok0:tok0 + 512], in0=k_x,
                                    in1=k_y, op=TT.subtract)
            nc.vector.tensor_tensor(out=krotz[32:64, tok0:tok0 + 512], in0=k_x2,
                                    in1=k_y2, op=TT.add)
            nc.gpsimd.tensor_copy(out=zkrot[64:128, tok0:tok0 + 512],
                                  in_=krotz[0:64, tok0:tok0 + 512])
            for k4 in range(4):
                tpv = kvp[:, 1, k4 * 32:(k4 + 1) * 32].bitcast(F16)
                nc.vector.tensor_copy(out=vnat[:, qt * 4 + k4, 0:64], in_=tpv)

        # ================= phase 2: attention (+ phase 3 interleaved) ======
        # yo tiles are emitted into the PV pair that is idle (the other
        # head-pair's accumulator), budgeted so the pair is free again before
        # the next (b, jq) needs it.
        pending = deque()   # (tt, nh) output tiles owed
        proj_ct = [0]
        drain = [False]  # in the drain loop ACT is idle: split casts across engines

        def emit_proj(pname):
            tt, nh = pending.popleft()
            yo = psum.tile([128, 2, 512], F32, name=pname)
            for half in range(2):
                n = nh * 2 + half
                nc.tensor.matmul(out=yo[:, half, :],
                                 lhsT=(a0[:, tt * 128:(tt + 1) * 128]),
                                 rhs=(wo_sb[:, 0, n * 512:(n + 1) * 512]),
                                 start=True, stop=False)
                nc.tensor.matmul(out=yo[:, half, :],
                                 lhsT=(a1[:, tt * 128:(tt + 1) * 128]),
                                 rhs=(wo_sb[:, 1, n * 512:(n + 1) * 512]),
                                 start=False, stop=True)
            stage = yst.tile([128, 2, 512], F16, name="stage")
            # in-loop casts stay on the DVE: a copy on ScalarE would queue
            # ahead of the next EXP in its FIFO and stall the score pipeline
            if drain[0] and proj_ct[0] % 2 == 1:
                nc.scalar.copy(out=stage, in_=yo)
            else:
                nc.vector.tensor_copy(out=stage, in_=yo)
            proj_ct[0] += 1
            nc.sync.dma_start(out=y[tt * 128:(tt + 1) * 128,
                                    nh * 1024:(nh + 1) * 1024], in_=stage)

        si = 0
        unit = 0
        for b in range(B):
            for jq in range(4):
                tq = b * S + jq * 512
                ni = 4 * jq + 4
                for pair in range(2):
                    pvp = psum.tile([65, 2, 512], F32, name=PNM[pair])
                    pend = None  # probs tile not yet fed to PV
                    for i in range(ni):
                        tk = b * S + i * 128
                        sc = psum.tile([128, 2, 512], F32, name=SNM[si % 2])
                        si += 1
                        for h2, kt in ((0, krotz), (1, zkrot)):
                            nc.tensor.matmul(
                                out=sc[:, h2, :], lhsT=(kt[:, tk:tk + 128]),
                                rhs=(qpack[:, pair, tq:tq + 512]),
                                start=True, stop=True)
                        if pend is not None:
                            ip, ptp = pend
                            for h2 in range(2):
                                nc.tensor.matmul(out=pvp[:, h2, :],
                                                 lhsT=(vnat[:, b * 16 + ip, :]),
                                                 rhs=(ptp[:, h2, :]),
                                                 start=ip == 0, stop=False)
                        if len(pending) > 2 and i % 2 == 1:
                            emit_proj(PNM[1 - pair])
                        pt = probs.tile([128, 2, 512], F16, name="pt")
                        nc.scalar.activation(out=pt, in_=sc,
                                             func=mybir.ActivationFunctionType.Exp,
                                             scale=float(SCALE))
                        unit += 1
                        if i >= 4 * jq:  # diagonal block: causal mask
                            roff = 128 * i - 512 * jq
                            nc.vector.tensor_tensor(
                                out=pt, in0=pt,
                                in1=mask2[:, :, 384 - roff:896 - roff], op=TT.mult)
                        pend = (i, pt)
                    ip, ptp = pend
                    for h2 in range(2):
                        nc.tensor.matmul(out=pvp[:, h2, :],
                                         lhsT=(vnat[:, b * 16 + ip, :]),
                                         rhs=(ptp[:, h2, :]),
                                         start=ip == 0, stop=True)
                    # normalize: row 64 of pvp is the softmax denominator.
                    # (the custom-DVE reciprocal reads garbage from PSUM on
                    # HW -- bounce the denominator row through SBUF first.)
                    sums = norm.tile([1, 2, 512], F32, name="sums")
                    nc.vector.tensor_copy(out=sums, in_=pvp[64:65, :, :])
                    rec = norm.tile([1, 2, 512], F32, name="rec")
                    nc.vector.reciprocal_approx_fast(out=rec, in_=sums)
                    dst = a0 if pair == 0 else a1
                    for h2 in range(2):
                        rbc = norm.tile([64, 512], F32, name="rbc")
                        nc.gpsimd.partition_broadcast(rbc, rec[0:1, h2, :])
                        nc.vector.tensor_tensor(out=dst[h2 * 64:h2 * 64 + 64, tq:tq + 512],
                                                in0=pvp[0:64, h2, :], in1=rbc,
                                                op=TT.mult)
                # output tiles of this (b, jq) are complete after both pairs
                tt0 = b * 16 + jq * 4
                pending.extend((tt0 + t, nh) for t in range(4) for nh in range(2))

        # remaining projections rotate through all four freed psum pairs
        drain[0] = True
        k = 0
        while pending:
            emit_proj((SNM + PNM)[k % 4])
            k += 1


_CACHE = {}


def _get_program():
    if "nc" not in _CACHE:
        _CACHE["nc"] = _build_program()
    return _CACHE["nc"]


def _get_runner():
    """Cached jitted shard_map executable over 8 cores (avoids per-call
    retrace that run_bass_kernel_spmd pays)."""
    if "runner" in _CACHE:
        return _CACHE["runner"]
    import jax
    from jax.sharding import Mesh, PartitionSpec
    from jax.experimental.shard_map import shard_map
    from concourse import bass2jax
    from concourse.bass2jax import _bass_exec_p

    bass2jax.install_neuronx_cc_hook()
    nc = _get_program()
    partition_name = nc.partition_id_tensor.name if nc.partition_id_tensor else None
    in_names, out_names, out_avals = [], [], []
    for alloc in nc.m.functions[0].allocations:
        if not isinstance(alloc, mybir.MemoryLocationSet):
            continue
        name = alloc.memorylocations[0].name
        if alloc.kind == "ExternalInput":
            if name != partition_name:
                in_names.append(name)
        elif alloc.kind == "ExternalOutput":
            out_names.append(name)
            out_avals.append(jax.core.ShapedArray(
                tuple(alloc.tensor_shape), mybir.dt.np(alloc.dtype)))
    n_params = len(in_names)
    n_outs = len(out_avals)
    all_in = list(in_names) + list(out_names)
    if partition_name is not None:
        all_in.append(partition_name)

    def _body_fn(*args):
        operands = list(args)
        if partition_name is not None:
            operands.append(bass2jax.partition_id_tensor())
        return tuple(_bass_exec_p.bind(
            *operands,
            out_avals=tuple(out_avals),
            in_names=tuple(all_in),
            out_names=tuple(out_names),
            lowering_input_output_aliases=(),
            sim_require_finite=True,
            sim_require_nnan=True,
            nc=nc,
        ))

    devices = jax.devices()[:NCORES]
    mesh = Mesh(np.asarray(devices), ("core",))
    # xT / rope tables / mask are identical on every core: feed them
    # replicated (P()) so the host uploads one copy + on-device all-gather,
    # instead of 8 copies through the tunnel.
    in_specs = tuple(
        PartitionSpec() if n in REPLICATED else PartitionSpec("core")
        for n in in_names) + (PartitionSpec("core"),) * n_outs
    sharded = jax.jit(
        shard_map(_body_fn, mesh=mesh,
                  in_specs=in_specs,
                  out_specs=(PartitionSpec("core"),) * n_outs,
                  check_rep=False),
        donate_argnums=tuple(range(n_params, n_params + n_outs)),
        keep_unused=True)

    from jax.sharding import NamedSharding
    rep = NamedSharding(mesh, PartitionSpec())
    shd = NamedSharding(mesh, PartitionSpec("core"))
    gather = jax.jit(lambda a: a, out_shardings=rep)   # upload-shard -> all-gather
    zeros = jax.jit(lambda: jnp.zeros((NCORES * T, D), jnp.float16),
                    out_shardings=shd)
    reduce_y = jax.jit(lambda yc: yc.reshape(NCORES, T, D)
                       .sum(0, dtype=jnp.float32), out_shardings=rep)
    _CACHE["runner"] = (sharded, in_names, out_names, out_avals,
                        mesh, rep, shd, gather, zeros, reduce_y)
    return _CACHE["runner"]


def _host_inputs(x, wq, wk, wv, wo):
    x = np.asarray(x, np.float32)
    wq = np.asarray(wq, np.float16)
    wk = np.asarray(wk, np.float16)
    wv = np.asarray(wv, np.float16)
    wo = np.asarray(wo, np.float16)

    xT = np.ascontiguousarray(x.reshape(T, D).T.astype(np.float16))

    inv = 1.0 / (THETA ** (np.arange(0, HD, 2, dtype=np.float64) / HD))
    fr = np.outer(inv, np.arange(S, dtype=np.float64))   # [32, S]
    cosT = np.cos(fr).astype(np.float32)
    sinT = np.sin(fr).astype(np.float32)
    cos4 = np.ascontiguousarray(np.tile(cosT, (4, 1)))
    sin4 = np.ascontiguousarray(np.tile(sinT, (4, 1)))

    u = np.arange(896)[None, :]
    p = np.arange(128)[:, None]
    mask = (u >= p + 384).astype(np.float16)

    in_maps = []
    for c in range(NCORES):
        cols_a, cols_b = [], []
        for h in range(HPC):
            base = (HPC * c + h) * HD
            cols_a.append(wq[:, base:base + 32])
            cols_b.append(wq[:, base + 32:base + 64])
        wq_c = np.ascontiguousarray(np.concatenate(cols_a + cols_b, axis=1))
        kb = c * HD
        wkv_c = np.ascontiguousarray(np.concatenate(
            [wk[:, kb:kb + 32], wk[:, kb + 32:kb + 64], wv[:, kb:kb + HD]], axis=1))
        wo_c = np.ascontiguousarray(wo[c * HPC * HD:(c + 1) * HPC * HD, :])
        in_maps.append({"xT": xT, "wq": wq_c, "wkv": wkv_c, "wo": wo_c,
                        "cos4": cos4, "sin4": sin4, "mask": mask})
    return in_maps


def _stage_inputs(in_maps):
    """Upload inputs: replicated tensors go up as 1/8 shards and are
    all-gathered on device; per-core tensors upload as the usual concat."""
    import jax
    (sharded, in_names, out_names, out_avals,
     mesh, rep, shd, gather, zeros, reduce_y) = _get_runner()
    staged = []
    for n in in_names:
        if n in REPLICATED:
            a = in_maps[0][n]
            if a.shape[0] % NCORES == 0:
                staged.append(gather(jax.device_put(a, shd)))
            else:
                staged.append(jax.device_put(a, rep))
        else:
            cat = np.concatenate([m[n] for m in in_maps], axis=0)
            staged.append(jax.device_put(cat, shd))
    return staged


def kernel(x, wq, wk, wv, wo):
    import jax
    (sharded, in_names, out_names, out_avals,
     mesh, rep, shd, gather, zeros, reduce_y) = _get_runner()
    in_maps = _host_inputs(x, wq, wk, wv, wo)
    staged = _stage_inputs(in_maps)
    out_arrs = sharded(*staged, zeros())
    ysum = reduce_y(out_arrs[out_names.index("y")])
    return np.asarray(ysum).reshape(B, S, D)
